# revision 1
# baseline (speedup 1.0000x reference)
"""Trainium2 Bass kernel for a pre-norm transformer block with dilated
windowed causal attention (B=2, L=2048, D=512, H=8, DIL=2, WIN=256,
HIDDEN=2048).

Sharding: 8 cores = batch(2) x sequence-chunk(4 x 512 tokens). Each core
receives its 512-token chunk plus a 256-token halo (keys/values only) and
computes the full block for its tokens; no collectives.

Device dataflow (per core):
  x [768,512] f32 -> LN1 (token-major stats) -> x_hat bf16 -> PE-transpose
  -> x_hat^T.  QKV (bf16 matmuls, fp32 psum): Q^T,K^T feature-major
  [hd, t]; V token-major per parity stream, ones-augmented per head.
  Attention per (head, parity stream, 128-query block): S^T[k,q] matmuls
  (dilation=2 -> two independent parity streams with a 128-wide causal
  window in stream coords), exp on ScalarE (no max subtraction: scores
  are O(1) here), band-mask multiply on GpSimd, PV with the ones row
  producing the softmax denominator, f32 reciprocal + gpsimd
  partition_broadcast for the normalization -> O^T feature-major bf16.
  out-proj -> residual (f32) -> LN2 -> FFN1+gelu -> FFN2 -> residual ->
  out [512,512] f32.

LayerNorm scale/bias are folded into the projection weights host-side;
weights are pre-transposed and cast to bf16 host-side.
"""
import os
import sys

os.environ.setdefault("MYCRO_LOCAL_CACHE", "1")
if "/opt/trn_rl_repo" not in sys.path:
    sys.path.insert(0, "/opt/trn_rl_repo")

import numpy as np

B, L, D, H, HD = 2, 2048, 512, 8, 64
HIDDEN = 4 * D
P = 128
CH = 512            # own tokens per core
HALO = 256
T = CH + HALO       # 768
NCORES = 8
EPS = 1e-5
SL = T // 2         # 384 keys per parity stream
SQ = CH // 2        # 256 queries per parity stream
SW = 128            # causal window in stream coords
SCALE = 1.0 / 8.0   # 1/sqrt(HD)

NT = T // P         # 6
NO = CH // P        # 4
ND = D // P         # 4
NHID = HIDDEN // P  # 16

_nc = None
LAST_EXEC_NS = None
LAST_RESULTS = None


def _body(ctx, tc, I, y):
    import concourse.bass as bass  # noqa: F401
    from concourse import mybir
    from concourse.masks import make_identity

    nc = tc.nc
    f32 = mybir.dt.float32
    bf16 = mybir.dt.bfloat16
    AF = mybir.ActivationFunctionType
    OP = mybir.AluOpType

    consts = ctx.enter_context(tc.tile_pool(name="consts", bufs=1))
    big = ctx.enter_context(tc.tile_pool(name="big", bufs=1))
    work = ctx.enter_context(tc.tile_pool(name="work", bufs=4))
    pmm = ctx.enter_context(tc.tile_pool(name="pmm", bufs=2, space="PSUM"))
    ptp = ctx.enter_context(tc.tile_pool(name="ptp", bufs=2, space="PSUM"))
    pa_s = ctx.enter_context(tc.tile_pool(name="pa_s", bufs=2, space="PSUM"))
    pa_o = ctx.enter_context(tc.tile_pool(name="pa_o", bufs=2, space="PSUM"))
    pexp = ctx.enter_context(tc.tile_pool(name="pexp", bufs=8))

    mm = nc.tensor.matmul

    def bcast(ap, p=P):
        return bass.AP(tensor=ap.tensor, offset=ap.offset,
                       ap=[[0, p]] + [list(d) for d in ap.ap])

    # ---------- constants ----------
    ident = consts.tile([P, P], bf16, tag="ident")
    make_identity(nc, ident)
    epst = consts.tile([P, 1], f32, tag="eps")
    nc.vector.memset(epst, EPS)
    esel = consts.tile([97, P], f32, tag="esel")
    nc.vector.memset(esel, 0.0)
    nc.vector.memset(esel[0:1, 0:64], 1.0)      # (stp0, hh0)
    nc.vector.memset(esel[64:65, 0:64], 1.0)    # (stp1, hh0)
    nc.vector.memset(esel[32:33, 64:128], 1.0)  # (stp0, hh1)
    nc.vector.memset(esel[96:97, 64:128], 1.0)  # (stp1, hh1)

    # PE warm-up: the PE is idle ~8us while x arrives; dummy matmuls keep
    # the HAM activity window busy so real work starts at full clock.
    junk = pmm.tile([P, P], f32, tag="ps")
    for _ in range(36):
        mm(junk, ident, ident, start=True, stop=True)

    x_sb = big.tile([P, NT, D], f32, tag="x")
    for c0 in range(0, NT, 2):
        nc.sync.dma_start(out=x_sb[:, c0:c0 + 2, :], in_=I["xc"][:, c0:c0 + 2, :])
    wqkv_sb = big.tile([P, ND, 3 * D], bf16, tag="w32a")
    nc.sync.dma_start(out=wqkv_sb, in_=I["wqkvT"])

    masks_sb = consts.tile([P, 3, 2 * SQ], bf16, tag="masks")
    nc.sync.dma_start(out=masks_sb, in_=I["masks"])
    bq_sb = consts.tile([P, 4], f32, tag="bq")
    nc.sync.dma_start(out=bq_sb, in_=I["bq"])
    bk_sb = consts.tile([P, 4], f32, tag="bk")
    nc.sync.dma_start(out=bk_sb, in_=I["bk"])
    b1_sb = consts.tile([P, NHID], f32, tag="b1")
    nc.sync.dma_start(out=b1_sb, in_=I["b1"])
    bv_sb = consts.tile([P, D], f32, tag="bv")
    nc.gpsimd.dma_start(out=bv_sb, in_=bcast(I["bv"]))
    bo_sb = consts.tile([P, D], f32, tag="bo")
    nc.gpsimd.dma_start(out=bo_sb, in_=bcast(I["bo"]))
    b2_sb = consts.tile([P, D], f32, tag="b2")
    nc.gpsimd.dma_start(out=b2_sb, in_=bcast(I["b2"]))

    # ---------- LN1 (token-major) ----------
    xhat = big.tile([P, NT, D], bf16, tag="t12a")
    for j in range(NT):
        st = work.tile([P, 6], f32, tag="bnst")
        nc.vector.bn_stats(st, x_sb[:, j, :])
        mv = work.tile([P, 2], f32, tag="bnmv")
        nc.vector.bn_aggr(mv, st)
        r = work.tile([P, 1], f32, tag="lnr")
        nc.scalar.activation(r, mv[:, 1:2], AF.Sqrt, bias=epst, scale=1.0)
        r2 = work.tile([P, 1], f32, tag="lnr2")
        nc.vector.reciprocal(r2, r)
        nc.vector.tensor_scalar(
            out=xhat[:, j, :], in0=x_sb[:, j, :],
            scalar1=mv[:, 0:1], scalar2=r2,
            op0=OP.subtract, op1=OP.mult,
        )

    # ---------- transpose x_hat -> x_hat^T [d, t] ----------
    xT = big.tile([P, ND, T], bf16, tag="t12b")
    for dt_ in range(ND):
        for j0 in range(0, NT, 2):
            pt = ptp.tile([P, 2 * P], bf16, tag="pt")
            for jj in range(2):
                nc.tensor.transpose(pt[:, jj * P:(jj + 1) * P],
                                    xhat[:, j0 + jj, dt_ * P:(dt_ + 1) * P], ident)
            nc.scalar.copy(xT[:, dt_, j0 * P:(j0 + 2) * P], pt)

    # pre-add the out-proj bias into the residual source during slack time
    for tt in range(NO):
        nc.vector.tensor_add(x_sb[:, 2 + tt, :], x_sb[:, 2 + tt, :], bo_sb)

    # ---------- QKV ----------

    # Q^T [o, own t]  (own tokens only)
    qT = big.tile([P, 4, CH], bf16, tag="t8a")
    for ot in range(4):
        ps = pmm.tile([P, CH], f32, tag="ps")
        for dt_ in range(ND):
            mm(ps, wqkv_sb[:, dt_, ot * P:(ot + 1) * P], xT[:, dt_, HALO:],
               start=(dt_ == 0), stop=(dt_ == ND - 1))
        nc.scalar.activation(qT[:, ot, :], ps, AF.Identity,
                             bias=bq_sb[:, ot:ot + 1], scale=1.0)

    # K^T [o, all t] in chunks of 512+256 (one PSUM bank each)
    kT = big.tile([P, 4, T], bf16, tag="t12a2")
    for ot in range(4):
        for c0, cn in ((0, 512), (512, 256)):
            ps = pmm.tile([P, CH], f32, tag="ps")
            for dt_ in range(ND):
                mm(ps[:, :cn], wqkv_sb[:, dt_, (4 + ot) * P:(5 + ot) * P],
                   xT[:, dt_, c0:c0 + cn],
                   start=(dt_ == 0), stop=(dt_ == ND - 1))
            nc.scalar.activation(kT[:, ot, c0:c0 + cn], ps[:, :cn], AF.Identity,
                                 bias=bk_sb[:, ot:ot + 1], scale=1.0)

    # V token-major per parity stream, ones-augmented per head:
    # v_sb[:, st*3+i, h, 0:64] = V tokens, [..., 64] = 1.0
    v_sb = big.tile([P, 6, H, 65], bf16, tag="t12c")
    for i in range(6):
        nc.vector.memset(v_sb[:, i, :, 64:65], 1.0)
    for stp in range(2):
        for i in range(3):
            ps = pmm.tile([P, D], f32, tag="ps")
            t0 = 2 * (i * P) + stp
            for dt_ in range(ND):
                mm(ps, xT[:, dt_, t0:t0 + 255:2], wqkv_sb[:, dt_, 2 * D:3 * D],
                   start=(dt_ == 0), stop=(dt_ == ND - 1))
            nc.vector.tensor_add(
                v_sb[:, stp * 3 + i, :, 0:64],
                ps.rearrange("p (h c) -> p h c", h=H),
                bv_sb.rearrange("p (h c) -> p h c", h=H),
            )

    # ---------- attention ----------
    # masks_sb[:, 0] = lower-tri (c<=r) with halo validity, qb=0 low tile
    # masks_sb[:, 1] = lower-tri (c<=r), qb=1 low tile
    # masks_sb[:, 2] = upper-tri (c>=r), high tiles
    oT = big.tile([P, 4, CH], bf16, tag="t12b2")
    oU = big.tile([P, 4, CH], bf16, tag="oU")
    den4s = {}
    for hp in range(4):
        den = work.tile([97, CH], f32, tag="den")
        den4s[hp] = den
        nc.vector.memset(den, 0.0)

    def emit_S_kt(hp, hh, kt, alt):
        # kt0 is only valid for the first 128 stream-queries, kt2 only for
        # the last 128 -- their tiles are half width (qw=128 per stream).
        lo = hh * 64
        qw = SQ if kt == 1 else P
        ps_s = pa_s.tile([P, 2 * SQ], f32, tag="ps_s")
        for stp in range(2):
            k0 = 2 * (kt * P) + stp
            q0 = stp if kt < 2 else 2 * P + stp
            mm(ps_s[:, stp * qw:(stp + 1) * qw],
               kT[lo:lo + 64, hp, k0:k0 + 255:2],
               qT[lo:lo + 64, hp, q0:q0 + 2 * qw - 1:2],
               start=True, stop=True)
        p_sb = pexp.tile([P, 2 * SQ], bf16, tag="p_sb")
        nc.scalar.activation(p_sb[:, :2 * qw], ps_s[:, :2 * qw],
                             AF.Exp, scale=SCALE)
        if kt == 0:
            nc.gpsimd.tensor_mul(p_sb[:, :2 * qw], p_sb[:, :2 * qw],
                                 masks_sb[:, kt, :2 * qw])
        elif kt == 1:
            nc.vector.tensor_mul(p_sb, p_sb, masks_sb[:, kt, :])
        elif alt:
            nc.gpsimd.tensor_mul(p_sb[:, :2 * qw], p_sb[:, :2 * qw],
                                 masks_sb[:, kt, :2 * qw])
        else:
            nc.vector.tensor_mul(p_sb[:, :2 * qw], p_sb[:, :2 * qw],
                                 masks_sb[:, kt, :2 * qw])
        return p_sb

    def emit_PV(hp, hh, p_sbs):
        h = 2 * hp + hh
        lo = hh * 64
        po = pa_o.tile([P, 2 * SQ], f32, tag="po")
        for stp in range(2):
            qa = stp * SQ             # first 128 queries of this stream
            qb = stp * SQ + P         # last 128 queries
            # region A: kt0 + kt1(first half); region B: kt1(second) + kt2
            mm(po[:65, qa:qa + P], v_sb[:, stp * 3 + 0, h, :],
               p_sbs[0][:, stp * P:(stp + 1) * P], start=True, stop=False)
            mm(po[:65, qa:qa + P], v_sb[:, stp * 3 + 1, h, :],
               p_sbs[1][:, stp * SQ:stp * SQ + P], start=False, stop=True)
            mm(po[:65, qb:qb + P], v_sb[:, stp * 3 + 1, h, :],
               p_sbs[1][:, stp * SQ + P:stp * SQ + 2 * P], start=True, stop=False)
            mm(po[:65, qb:qb + P], v_sb[:, stp * 3 + 2, h, :],
               p_sbs[2][:, stp * P:(stp + 1) * P], start=False, stop=True)
        den = den4s[hp]
        for stp in range(2):
            nc.vector.tensor_copy(oU[lo:lo + 64, hp, stp::2],
                                  po[:64, stp * SQ:(stp + 1) * SQ])
            k_ = 32 * (2 * stp + hh)
            nc.vector.tensor_copy(den[k_:k_ + 1, stp::2],
                                  po[64:65, stp * SQ:(stp + 1) * SQ])

    def emit_norm(hp):
        pb = pmm.tile([P, CH], f32, tag="ps")
        mm(pb, esel, den4s[hp], start=True, stop=True)
        rb = work.tile([P, CH], f32, tag="rb")
        scr = work.tile([P, CH], f32, tag="rbscr")
        nc.vector.reciprocal_approx_accurate(rb, pb, scr)
        nc.vector.tensor_mul(oT[:, hp, :], oU[:, hp, :], rb)

    chains = [(hp, hh) for hp in range(4) for hh in range(2)]
    prev = None
    for ci, (hp, hh) in enumerate(chains):
        alt = ci % 2 == 0
        p_sbs = [emit_S_kt(hp, hh, 0, alt), emit_S_kt(hp, hh, 1, alt)]
        if prev is not None:
            emit_PV(*prev)
            if prev[1] == 1:          # second chain of prev[0] done
                emit_norm(prev[0])
        p_sbs.append(emit_S_kt(hp, hh, 2, alt))
        prev = (hp, hh, p_sbs)
    emit_PV(*prev)
    emit_norm(prev[0])

    # ---------- out projection + residual ----------
    wo_sb = big.tile([P, ND, D], bf16, tag="t8b")
    nc.sync.dma_start(out=wo_sb, in_=I["woT"])
    res1 = big.tile([P, NO, D], f32, tag="t12c2")
    for tt in range(NO):
        ps = pmm.tile([P, D], f32, tag="ps")
        for dt_ in range(ND):
            mm(ps, oT[:, dt_, tt * P:(tt + 1) * P], wo_sb[:, dt_, :],
               start=(dt_ == 0), stop=(dt_ == ND - 1))
        nc.vector.tensor_add(res1[:, tt, :], ps, x_sb[:, 2 + tt, :])

    # ---------- LN2 ----------
    xhat2 = big.tile([P, NO, D], bf16, tag="t8b2")
    for j in range(NO):
        st = work.tile([P, 6], f32, tag="bnst")
        nc.vector.bn_stats(st, res1[:, j, :])
        mv = work.tile([P, 2], f32, tag="bnmv")
        nc.vector.bn_aggr(mv, st)
        r = work.tile([P, 1], f32, tag="lnr")
        nc.scalar.activation(r, mv[:, 1:2], AF.Sqrt, bias=epst, scale=1.0)
        r2 = work.tile([P, 1], f32, tag="lnr2")
        nc.vector.reciprocal(r2, r)
        nc.vector.tensor_scalar(
            out=xhat2[:, j, :], in0=res1[:, j, :],
            scalar1=mv[:, 0:1], scalar2=r2,
            op0=OP.subtract, op1=OP.mult,
        )

    for tt in range(NO):
        nc.vector.tensor_add(res1[:, tt, :], res1[:, tt, :], b2_sb)

    x2T = big.tile([P, ND, CH], bf16, tag="t8a2")
    for j in range(NO):
        for dt_ in range(ND):
            pt = ptp.tile([P, 2 * P], bf16, tag="pt")
            nc.tensor.transpose(pt[:, 0:P],
                                xhat2[:, j, dt_ * P:(dt_ + 1) * P], ident)
            nc.scalar.copy(x2T[:, dt_, j * P:(j + 1) * P], pt[:, 0:P])

    # ---------- FFN1 (+gelu), feature-major G^T [h, t] ----------
    w1_sb = big.tile([P, ND, HIDDEN], bf16, tag="w32a2")
    nc.sync.dma_start(out=w1_sb, in_=I["w1T"])
    g_sb = big.tile([P, NHID, CH], bf16, tag="g32")
    for ht in range(NHID):
        ps = pmm.tile([P, CH], f32, tag="ps")
        for dt_ in range(ND):
            mm(ps, w1_sb[:, dt_, ht * P:(ht + 1) * P], x2T[:, dt_, :],
               start=(dt_ == 0), stop=(dt_ == ND - 1))
        nc.scalar.activation(g_sb[:, ht, :], ps, AF.Gelu,
                             bias=b1_sb[:, ht:ht + 1], scale=1.0)

    # ---------- FFN2 + residual ----------
    w2_sb = big.tile([P, NHID, D], bf16, tag="w32b")
    nc.sync.dma_start(out=w2_sb, in_=I["w2T"])
    fin = big.tile([P, NO, D], f32, tag="t8b3")
    for tt in range(NO):
        ps = pmm.tile([P, D], f32, tag="ps")
        for ht in range(NHID):
            mm(ps, g_sb[:, ht, tt * P:(tt + 1) * P], w2_sb[:, ht, :],
               start=(ht == 0), stop=(ht == NHID - 1))
        nc.vector.tensor_add(fin[:, tt, :], ps, res1[:, tt, :])

    yr = y.rearrange("(j p) d -> p j d", p=P)
    for tt in range(NO):
        nc.sync.dma_start(out=yr[:, tt, :], in_=fin[:, tt, :])


def _build():
    from contextlib import ExitStack

    import concourse.bacc as bacc
    import concourse.tile as tile
    from concourse import mybir

    f32 = mybir.dt.float32
    bf16 = mybir.dt.bfloat16
    nc = bacc.Bacc("TRN2", target_bir_lowering=False, debug=False,
                   enable_asserts=False, num_devices=NCORES)
    I = {}

    def inp(name, shape, dt_):
        I[name] = nc.dram_tensor(name, list(shape), dt_, kind="ExternalInput").ap()

    inp("xc", (P, NT, D), f32)
    inp("wqkvT", (P, ND, 3 * D), bf16)
    inp("bq", (P, 4), f32)
    inp("bk", (P, 4), f32)
    inp("bv", (D,), f32)
    inp("woT", (P, ND, D), bf16)
    inp("bo", (D,), f32)
    inp("w1T", (P, ND, HIDDEN), bf16)
    inp("b1", (P, NHID), f32)
    inp("w2T", (P, NHID, D), bf16)
    inp("b2", (D,), f32)
    inp("masks", (P, 3, 2 * SQ), bf16)
    y = nc.dram_tensor("y", [CH, D], f32, kind="ExternalOutput").ap()

    with tile.TileContext(nc) as tc:
        with ExitStack() as ctx:
            _body(ctx, tc, I, y)
    nc.compile()
    return nc


def _host_masks():
    import ml_dtypes
    sk = np.arange(SL)[:, None]
    sq = np.arange(SL - SQ, SL)[None, :]
    valid = ((sq - sk >= 0) & (sq - sk <= SW)).astype(np.float32)  # [384, 256]
    z = np.zeros((P, SQ), np.float32)
    kt0 = valid[0:P, 0:P]           # lower-tri; only first 128 queries valid
    kt1 = valid[P:2 * P, :]         # full band
    kt2 = valid[2 * P:3 * P, P:SQ]  # upper-tri; only last 128 queries valid
    m = np.stack([
        np.concatenate([kt0, kt0, z[:, :0]], 1) if False else np.concatenate([kt0, kt0, np.zeros((P, SQ), np.float32)], 1),
        np.concatenate([kt1, kt1], 1),
        np.concatenate([kt2, kt2, np.zeros((P, SQ), np.float32)], 1),
    ]).astype(ml_dtypes.bfloat16)
    m0 = m.copy()
    m0[0] = 0.0  # first chunk of each batch: halo keys invalid
    m = np.ascontiguousarray(m.transpose(1, 0, 2))
    m0 = np.ascontiguousarray(m0.transpose(1, 0, 2))
    return m, m0


def get_nc():
    global _nc
    if _nc is None:
        _nc = _build()
    return _nc


def _pmaj(a, p=P):
    """[N*p, F...] row-major -> [p, N, F...] partition-major contiguous."""
    n = a.shape[0] // p
    return np.ascontiguousarray(
        a.reshape((n, p) + a.shape[1:]).transpose((1, 0) + tuple(range(2, a.ndim + 1))))


def make_in_maps(inputs):
    import ml_dtypes
    f = np.float32
    bf = ml_dtypes.bfloat16
    x = np.asarray(inputs["x"], f)
    qkv_w = np.asarray(inputs["qkv_w"], f)
    n1w = np.asarray(inputs["norm1_w"], f)
    n1b = np.asarray(inputs["norm1_b"], f)
    wqkv_f = qkv_w * n1w[None, :]
    bqkv = qkv_w @ n1b + np.asarray(inputs["qkv_b"], f)
    wqkvT = _pmaj(np.ascontiguousarray(wqkv_f.T).astype(bf))
    bq = np.ascontiguousarray(bqkv[0:D].reshape(4, P).T)
    bk = np.ascontiguousarray(bqkv[D:2 * D].reshape(4, P).T)
    bv = np.ascontiguousarray(bqkv[2 * D:3 * D])

    woT = _pmaj(np.ascontiguousarray(np.asarray(inputs["out_w"], f).T).astype(bf))
    bo = np.ascontiguousarray(np.asarray(inputs["out_b"], f))

    w1 = np.asarray(inputs["ffn_w1"], f)
    n2w = np.asarray(inputs["norm2_w"], f)
    n2b = np.asarray(inputs["norm2_b"], f)
    w1T = _pmaj(np.ascontiguousarray((w1 * n2w[None, :]).T).astype(bf))
    b1v = w1 @ n2b + np.asarray(inputs["ffn_b1"], f)
    b1 = np.ascontiguousarray(b1v.reshape(NHID, P).T)
    w2T = _pmaj(np.ascontiguousarray(np.asarray(inputs["ffn_w2"], f).T).astype(bf))
    b2 = np.ascontiguousarray(np.asarray(inputs["ffn_b2"], f))

    masks, masks0 = _host_masks()
    shared = dict(wqkvT=wqkvT, bq=bq, bk=bk, bv=bv, woT=woT, bo=bo,
                  w1T=w1T, b1=b1, w2T=w2T, b2=b2)
    in_maps = []
    for c in range(NCORES):
        b_, i = divmod(c, 4)
        own = x[b_, i * CH:(i + 1) * CH]
        if i == 0:
            halo = np.zeros((HALO, D), f)
        else:
            halo = x[b_, i * CH - HALO:i * CH]
        xc = _pmaj(np.concatenate([halo, own], 0))
        in_maps.append(dict(xc=xc, masks=(masks if i > 0 else masks0), **shared))
    return in_maps


def kernel(**inputs):
    global LAST_EXEC_NS, LAST_RESULTS
    from concourse.bass_utils import run_bass_kernel_spmd

    nc = get_nc()
    in_maps = make_in_maps(inputs)
    trace = bool(int(os.environ.get("BASS_KERNEL_TRACE", "0")))
    res = run_bass_kernel_spmd(nc, in_maps, core_ids=list(range(NCORES)),
                               trace=trace)
    LAST_EXEC_NS = res.exec_time_ns
    LAST_RESULTS = res
    out = np.zeros((B, L, D), np.float32)
    for c, r in enumerate(res.results):
        b_, i = divmod(c, 4)
        out[b_, i * CH:(i + 1) * CH] = r["y"]
    return out



# revision 2
# speedup vs baseline: 1.0172x; 1.0172x over previous
"""Trainium2 Bass kernel for a pre-norm transformer block with dilated
windowed causal attention (B=2, L=2048, D=512, H=8, DIL=2, WIN=256,
HIDDEN=2048).

Sharding: 8 cores = batch(2) x sequence-chunk(4 x 512 tokens). Each core
receives its 512-token chunk plus a 256-token halo (keys/values only) and
computes the full block for its tokens; no collectives.

Token axis is STREAM-MAJOR (dilation parity streams separated, host
reorders): t = [s0 own 256 | s1 own 256 | s0 halo 128 | s1 halo 128].
All attention slices are contiguous; host un-permutes the output.

Projection matmuls (QKV / out-proj / FFN) run in fp8e4 DoubleRow (two
128-deep k-planes per instruction). Weights are scaled x64 into fp8
range host-side; the 1/64 (1/1024 with the x16 oT scale) is folded into
the PSUM evacuation. Attention S/PV matmuls stay bf16. Softmax
denominator rides the PV matmul as a ones-row; its reciprocal is taken
straight out of PSUM into a bf16 row and broadcast across partitions
with a tiny bf16 matmul.
"""
import os
import sys

os.environ.setdefault("MYCRO_LOCAL_CACHE", "1")
if "/opt/trn_rl_repo" not in sys.path:
    sys.path.insert(0, "/opt/trn_rl_repo")

import numpy as np

B, L, D, H, HD = 2, 2048, 512, 8, 64
HIDDEN = 4 * D
P = 128
CH = 512            # own tokens per core
HALO = 256
T = CH + HALO       # 768
NCORES = 8
EPS = 1e-5
SQ = 256            # own queries per parity stream
SCALE = 1.0 / 8.0   # 1/sqrt(HD)
WS = 64.0           # host-side fp8 weight scale
OS = 16.0           # oT fp8 scale (folded into esel)

NT = T // P         # 6
NO = CH // P        # 4
ND = D // P         # 4
NHID = HIDDEN // P  # 16

_nc = None
LAST_EXEC_NS = None
LAST_RESULTS = None


def _body(ctx, tc, I, y):
    import concourse.bass as bass  # noqa: F401
    from concourse import mybir

    nc = tc.nc
    f32 = mybir.dt.float32
    bf16 = mybir.dt.bfloat16
    f8 = mybir.dt.float8e4
    AF = mybir.ActivationFunctionType
    OP = mybir.AluOpType
    DR = mybir.MatmulPerfMode.DoubleRow

    consts = ctx.enter_context(tc.tile_pool(name="consts", bufs=1))
    big = ctx.enter_context(tc.tile_pool(name="big", bufs=1))
    work = ctx.enter_context(tc.tile_pool(name="work", bufs=4))
    pmm = ctx.enter_context(tc.tile_pool(name="pmm", bufs=2, space="PSUM"))
    ptp = ctx.enter_context(tc.tile_pool(name="ptp", bufs=2, space="PSUM"))
    pa_s = ctx.enter_context(tc.tile_pool(name="pa_s", bufs=2, space="PSUM"))
    pa_o = ctx.enter_context(tc.tile_pool(name="pa_o", bufs=2, space="PSUM"))
    pexp = ctx.enter_context(tc.tile_pool(name="pexp", bufs=8))

    mm = nc.tensor.matmul

    def bcast(ap, p=P):
        return bass.AP(tensor=ap.tensor, offset=ap.offset,
                       ap=[[0, p]] + [list(d) for d in ap.ap])

    # ---------- input DMAs (x first: critical path; weights overlap) ----
    ident = consts.tile([P, P], bf16, tag="ident")
    nc.sync.dma_start(out=ident, in_=I["ident"])
    x_sb = big.tile([P, NT, D], f32, tag="x")
    for c0 in range(0, NT, 2):
        nc.sync.dma_start(out=x_sb[:, c0:c0 + 2, :], in_=I["xc"][:, c0:c0 + 2, :])
    masks_sb = consts.tile([P, 3, 2 * SQ], bf16, tag="masks")
    nc.sync.dma_start(out=masks_sb, in_=I["masks"])
    bq_sb = consts.tile([P, 4], f32, tag="bq")
    nc.sync.dma_start(out=bq_sb, in_=I["bq"])
    bk_sb = consts.tile([P, 4], f32, tag="bk")
    nc.sync.dma_start(out=bk_sb, in_=I["bk"])
    b1_sb = consts.tile([P, NHID], f32, tag="b1")
    nc.sync.dma_start(out=b1_sb, in_=I["b1"])
    wqkv_sb = big.tile([P, ND, 3 * D], f8, tag="wqkv")
    nc.sync.dma_start(out=wqkv_sb, in_=I["wqkvT"])
    wo_sb = big.tile([P, ND, D], f8, tag="wo")
    nc.sync.dma_start(out=wo_sb, in_=I["woT"])
    w1_sb = big.tile([P, ND, HIDDEN], f8, tag="w1")
    nc.sync.dma_start(out=w1_sb, in_=I["w1T"])
    w2_sb = big.tile([P, NHID, D], f8, tag="w2")
    nc.sync.dma_start(out=w2_sb, in_=I["w2T"])
    bo_sb = consts.tile([P, D], f32, tag="bo")
    nc.gpsimd.dma_start(out=bo_sb, in_=bcast(I["bo"]))
    b2_sb = consts.tile([P, D], f32, tag="b2")
    nc.gpsimd.dma_start(out=b2_sb, in_=bcast(I["b2"]))

    epst = consts.tile([P, 1], f32, tag="eps")
    nc.vector.memset(epst, EPS)
    esel = consts.tile([65, P], bf16, tag="esel")
    nc.vector.memset(esel, 0.0)
    nc.vector.memset(esel[0:1, 0:64], OS)    # hh=0 row
    nc.vector.memset(esel[64:65, 64:128], OS)  # hh=1 row

    # PE warm-up: dummy matmuls keep the HAM activity window busy so real
    # work starts at full clock.
    junk = pmm.tile([P, 512], f32, tag="ps")
    for _ in range(30):
        mm(junk[:, :P], ident, ident, start=True, stop=True)

    # ---------- LN1 (token-major stats; apply on ScalarE) ----------
    xhat = big.tile([P, NT, D], bf16, tag="xhat")

    def emit_ln(src, dst):
        st = work.tile([P, 6], f32, tag="bnst")
        nc.vector.bn_stats(st, src)
        mv = work.tile([P, 2], f32, tag="bnmv")
        nc.vector.bn_aggr(mv, st)
        r = work.tile([P, 1], f32, tag="lnr")
        nc.scalar.activation(r, mv[:, 1:2], AF.Sqrt, bias=epst, scale=1.0)
        r2 = work.tile([P, 1], f32, tag="lnr2")
        nc.vector.reciprocal(r2, r)
        nb = work.tile([P, 1], f32, tag="lnnb")
        nc.vector.tensor_scalar(out=nb, in0=mv[:, 0:1], scalar1=r2,
                                scalar2=-1.0, op0=OP.mult, op1=OP.mult)
        nc.scalar.activation(dst, src, AF.Identity, bias=nb, scale=r2)

    # transpose chunk-pair cp of xhat into xT (fp8), PE transpose + copy
    xT = big.tile([P, ND, T], f8, tag="xT")

    def emit_tp(cp):
        for dt_ in range(ND):
            pt = ptp.tile([P, 2 * P], bf16, tag="pt")
            for jj in range(2):
                nc.tensor.transpose(pt[:, jj * P:(jj + 1) * P],
                                    xhat[:, 2 * cp + jj, dt_ * P:(dt_ + 1) * P],
                                    ident)
            nc.scalar.copy(xT[:, dt_, cp * 2 * P:(cp + 1) * 2 * P], pt)

    kT = big.tile([P, 4, T], bf16, tag="kT")

    def emit_k(kc):  # K over token chunk [kc*256, (kc+1)*256)
        t0 = kc * 256
        for ot in range(4):
            ps = pmm.tile([P, 512], f32, tag="ps")
            for dp in range(2):
                mm(ps[:, :256], wqkv_sb[:, 2 * dp:2 * dp + 2, (4 + ot) * P:(5 + ot) * P],
                   xT[:, 2 * dp:2 * dp + 2, t0:t0 + 256],
                   start=(dp == 0), stop=(dp == 1), perf_mode=DR)
            nc.scalar.activation(kT[:, ot, t0:t0 + 256], ps[:, :256], AF.Identity,
                                 bias=bk_sb[:, ot:ot + 1], scale=1.0 / WS)

    qT = big.tile([P, 4, CH], bf16, tag="qT")

    def emit_q():
        for ot in range(4):
            ps = pmm.tile([P, 512], f32, tag="ps")
            for dp in range(2):
                mm(ps, wqkv_sb[:, 2 * dp:2 * dp + 2, ot * P:(ot + 1) * P],
                   xT[:, 2 * dp:2 * dp + 2, 0:CH],
                   start=(dp == 0), stop=(dp == 1), perf_mode=DR)
            nc.scalar.activation(qT[:, ot, :], ps, AF.Identity,
                                 bias=bq_sb[:, ot:ot + 1], scale=1.0 / WS)

    # V token-major per (stream, kt) block, ones-augmented per head
    v_sb = big.tile([P, 6, H, 65], bf16, tag="v")
    # key-tile -> t-range base: blk = s*3 + kt
    KB = {(0, 0): 512, (0, 1): 0, (0, 2): 128,
          (1, 0): 640, (1, 1): 256, (1, 2): 384}

    def emit_v(s, kt):
        t0 = KB[(s, kt)]
        blk = s * 3 + kt
        ps = pmm.tile([P, 512], f32, tag="ps")
        for dp in range(2):
            mm(ps, xT[:, 2 * dp:2 * dp + 2, t0:t0 + P],
               wqkv_sb[:, 2 * dp:2 * dp + 2, 2 * D:3 * D],
               start=(dp == 0), stop=(dp == 1), perf_mode=DR)
        nc.scalar.mul(v_sb[:, blk, :, 0:64],
                      ps.rearrange("p (h c) -> p h c", h=H), 1.0 / WS)

    for i in range(6):
        nc.vector.memset(v_sb[:, i, :, 64:65], 1.0)

    # pre-add the out-proj bias into the residual source during slack time
    for tt in range(NO):
        nc.gpsimd.tensor_add(x_sb[:, tt, :], x_sb[:, tt, :], bo_sb)

    # head phase: LN chunk-pairs -> transpose -> K/Q/V as deps allow
    emit_ln(x_sb[:, 0, :], xhat[:, 0, :])
    emit_ln(x_sb[:, 1, :], xhat[:, 1, :])
    emit_tp(0)
    emit_ln(x_sb[:, 2, :], xhat[:, 2, :])
    emit_ln(x_sb[:, 3, :], xhat[:, 3, :])
    emit_tp(1)
    emit_k(0)
    emit_ln(x_sb[:, 4, :], xhat[:, 4, :])
    emit_ln(x_sb[:, 5, :], xhat[:, 5, :])
    emit_tp(2)
    emit_k(1)
    emit_q()
    emit_v(0, 1)
    emit_v(0, 2)
    emit_v(1, 1)
    emit_v(1, 2)
    emit_k(2)
    emit_v(0, 0)
    emit_v(1, 0)

    # ---------- attention ----------
    # masks_sb[:, 0, :256] = [tril|tril]  (kt0; zeroed for first chunk)
    # masks_sb[:, 1, :512] = [triu|tril|triu|tril]  (kt1)
    # masks_sb[:, 2, :256] = [triu|triu]  (kt2)
    oU = big.tile([P, 4, CH], bf16, tag="oU")
    oT = big.tile([P, 4, CH], f8, tag="oT")
    den4s = {}
    for hp in range(4):
        den = work.tile([65, CH], bf16, tag="den")
        den4s[hp] = den
        nc.vector.memset(den, 1.0)

    def emit_S_kt(hp, hh, kt, alt):
        lo = hh * 64
        qw = 2 * SQ if kt == 1 else SQ
        ps_s = pa_s.tile([P, 2 * SQ], f32, tag="ps_s")
        for s in range(2):
            k0 = KB[(s, kt)]
            q0 = s * SQ if kt < 2 else s * SQ + P
            w = qw // 2
            mm(ps_s[:, s * w:(s + 1) * w],
               kT[lo:lo + 64, hp, k0:k0 + P],
               qT[lo:lo + 64, hp, q0:q0 + w],
               start=True, stop=True)
        p_sb = pexp.tile([P, 2 * SQ], bf16, tag="p_sb")
        nc.scalar.activation(p_sb[:, :qw], ps_s[:, :qw], AF.Exp, scale=SCALE)
        if kt == 1 and not alt:
            nc.vector.tensor_mul(p_sb, p_sb, masks_sb[:, kt, :])
        else:
            nc.gpsimd.tensor_mul(p_sb[:, :qw], p_sb[:, :qw],
                                 masks_sb[:, kt, :qw])
        return p_sb

    def emit_PV(hp, hh, p_sbs):
        h = 2 * hp + hh
        lo = hh * 64
        po = pa_o.tile([P, 2 * SQ], f32, tag="po")
        for s in range(2):
            qa = s * SQ
            qb = s * SQ + P
            mm(po[:65, qa:qa + P], v_sb[:, s * 3 + 0, h, :],
               p_sbs[0][:, s * P:(s + 1) * P], start=True, stop=False)
            mm(po[:65, qa:qa + P], v_sb[:, s * 3 + 1, h, :],
               p_sbs[1][:, s * 2 * P:s * 2 * P + P], start=False, stop=True)
            mm(po[:65, qb:qb + P], v_sb[:, s * 3 + 1, h, :],
               p_sbs[1][:, s * 2 * P + P:(s + 1) * 2 * P], start=True, stop=False)
            mm(po[:65, qb:qb + P], v_sb[:, s * 3 + 2, h, :],
               p_sbs[2][:, s * P:(s + 1) * P], start=False, stop=True)
        if hh == 0:
            nc.scalar.copy(oU[lo:lo + 64, hp, :], po[:64, :])
        else:
            nc.vector.tensor_copy(oU[lo:lo + 64, hp, :], po[:64, :])
        with nc.allow_low_precision("softmax denominator in bf16"):
            nc.vector.reciprocal(den4s[hp][lo:lo + 1, :], po[64:65, :])

    def emit_norm(hp):
        pb = pmm.tile([P, 512], f32, tag="ps")
        mm(pb, esel, den4s[hp], start=True, stop=True)
        nc.vector.tensor_mul(oT[:, hp, :], oU[:, hp, :], pb)

    chains = [(hp, hh) for hp in range(4) for hh in range(2)]
    prev = None
    for ci, (hp, hh) in enumerate(chains):
        alt = ci % 2 == 0
        p_sbs = [emit_S_kt(hp, hh, 0, alt), emit_S_kt(hp, hh, 1, alt)]
        if prev is not None:
            emit_PV(*prev)
            if prev[1] == 1:
                emit_norm(prev[0])
        p_sbs.append(emit_S_kt(hp, hh, 2, alt))
        prev = (hp, hh, p_sbs)
    emit_PV(*prev)
    emit_norm(prev[0])

    # ---------- out projection + residual ----------
    res1 = big.tile([P, NO, D], f32, tag="res1")
    for tt in range(NO):
        ps = pmm.tile([P, 512], f32, tag="ps")
        for pp in range(2):
            mm(ps, oT[:, 2 * pp:2 * pp + 2, tt * P:(tt + 1) * P],
               wo_sb[:, 2 * pp:2 * pp + 2, :],
               start=(pp == 0), stop=(pp == 1), perf_mode=DR)
        nc.vector.scalar_tensor_tensor(out=res1[:, tt, :], in0=ps,
                                       scalar=1.0 / (WS * OS),
                                       in1=x_sb[:, tt, :],
                                       op0=OP.mult, op1=OP.add)

    # ---------- LN2 + transpose to x2T (fp8) ----------
    xhat2 = big.tile([P, NO, D], bf16, tag="xhat2")
    for j in range(NO):
        emit_ln(res1[:, j, :], xhat2[:, j, :])
    for tt in range(NO):
        nc.vector.tensor_add(res1[:, tt, :], res1[:, tt, :], b2_sb)

    x2T = big.tile([P, ND, CH], f8, tag="x2T")
    for dt_ in range(ND):
        for cp in range(2):
            pt = ptp.tile([P, 2 * P], bf16, tag="pt")
            for jj in range(2):
                nc.tensor.transpose(pt[:, jj * P:(jj + 1) * P],
                                    xhat2[:, 2 * cp + jj, dt_ * P:(dt_ + 1) * P],
                                    ident)
            nc.scalar.copy(x2T[:, dt_, cp * 2 * P:(cp + 1) * 2 * P], pt)

    # ---------- FFN1 (+gelu) -> G^T fp8 [h, t] ----------
    g_sb = big.tile([P, NHID, CH], f8, tag="g")
    for ht in range(NHID):
        ps = pmm.tile([P, 512], f32, tag="ps")
        for dp in range(2):
            mm(ps, w1_sb[:, 2 * dp:2 * dp + 2, ht * P:(ht + 1) * P],
               x2T[:, 2 * dp:2 * dp + 2, :],
               start=(dp == 0), stop=(dp == 1), perf_mode=DR)
        nc.scalar.activation(g_sb[:, ht, :], ps, AF.Gelu,
                             bias=b1_sb[:, ht:ht + 1], scale=1.0 / WS)

    # ---------- FFN2 + residual ----------
    fin = big.tile([P, NO, D], f32, tag="fin")
    for tt in range(NO):
        ps = pmm.tile([P, 512], f32, tag="ps")
        for hp in range(NHID // 2):
            mm(ps, g_sb[:, 2 * hp:2 * hp + 2, tt * P:(tt + 1) * P],
               w2_sb[:, 2 * hp:2 * hp + 2, :],
               start=(hp == 0), stop=(hp == NHID // 2 - 1), perf_mode=DR)
        nc.vector.scalar_tensor_tensor(out=fin[:, tt, :], in0=ps,
                                       scalar=1.0 / WS, in1=res1[:, tt, :],
                                       op0=OP.mult, op1=OP.add)

    yr = y.rearrange("(j p) d -> p j d", p=P)
    for tt in range(NO):
        nc.sync.dma_start(out=yr[:, tt, :], in_=fin[:, tt, :])


def _build():
    from contextlib import ExitStack

    import concourse.bacc as bacc
    import concourse.tile as tile
    from concourse import mybir

    f32 = mybir.dt.float32
    bf16 = mybir.dt.bfloat16
    f8 = mybir.dt.float8e4
    nc = bacc.Bacc("TRN2", target_bir_lowering=False, debug=False,
                   enable_asserts=False, num_devices=NCORES)
    I = {}

    def inp(name, shape, dt_):
        I[name] = nc.dram_tensor(name, list(shape), dt_, kind="ExternalInput").ap()

    inp("xc", (P, NT, D), f32)
    inp("ident", (P, P), bf16)
    inp("wqkvT", (P, ND, 3 * D), f8)
    inp("bq", (P, 4), f32)
    inp("bk", (P, 4), f32)
    inp("woT", (P, ND, D), f8)
    inp("bo", (D,), f32)
    inp("w1T", (P, ND, HIDDEN), f8)
    inp("b1", (P, NHID), f32)
    inp("w2T", (P, NHID, D), f8)
    inp("b2", (D,), f32)
    inp("masks", (P, 3, 2 * SQ), bf16)
    y = nc.dram_tensor("y", [CH, D], f32, kind="ExternalOutput").ap()

    with tile.TileContext(nc) as tc:
        with ExitStack() as ctx:
            _body(ctx, tc, I, y)
    nc.compile()
    return nc


def _host_masks():
    import ml_dtypes
    tril = np.tril(np.ones((P, P), np.float32))
    triu = np.triu(np.ones((P, P), np.float32))
    z = np.zeros((P, P), np.float32)
    kt0 = np.concatenate([tril, tril, z, z], 1)
    kt1 = np.concatenate([triu, tril, triu, tril], 1)
    kt2 = np.concatenate([triu, triu, z, z], 1)
    m = np.stack([kt0, kt1, kt2]).astype(ml_dtypes.bfloat16)
    m0 = m.copy()
    m0[0] = 0.0  # first chunk of each batch: halo keys invalid
    m = np.ascontiguousarray(m.transpose(1, 0, 2))
    m0 = np.ascontiguousarray(m0.transpose(1, 0, 2))
    return m, m0


def get_nc():
    global _nc
    if _nc is None:
        _nc = _build()
    return _nc


def _pmaj(a, p=P):
    """[N*p, F...] row-major -> [p, N, F...] partition-major contiguous."""
    n = a.shape[0] // p
    return np.ascontiguousarray(
        a.reshape((n, p) + a.shape[1:]).transpose((1, 0) + tuple(range(2, a.ndim + 1))))


# stream-major chunk gather: full[768] = [halo 256 | own 512]
_a = np.arange(128)
_XIDX = np.concatenate([256 + 2 * _a, 512 + 2 * _a, 257 + 2 * _a,
                        513 + 2 * _a, 2 * _a, 1 + 2 * _a])
# y rows (chunk-major stream order) -> original own-token index
_TOKPERM = np.concatenate([2 * _a, 256 + 2 * _a, 1 + 2 * _a, 257 + 2 * _a])


def make_in_maps(inputs):
    import ml_dtypes
    f = np.float32
    bf = ml_dtypes.bfloat16
    f8 = ml_dtypes.float8_e4m3
    x = np.asarray(inputs["x"], f)
    qkv_w = np.asarray(inputs["qkv_w"], f)
    n1w = np.asarray(inputs["norm1_w"], f)
    n1b = np.asarray(inputs["norm1_b"], f)
    wqkv_f = qkv_w * n1w[None, :]
    bqkv = qkv_w @ n1b + np.asarray(inputs["qkv_b"], f)
    wqkvT = _pmaj(np.ascontiguousarray(wqkv_f.T) * WS).astype(f8)
    bq = np.ascontiguousarray(bqkv[0:D].reshape(4, P).T)
    bk = np.ascontiguousarray(bqkv[D:2 * D].reshape(4, P).T)
    bv = np.ascontiguousarray(bqkv[2 * D:3 * D])

    out_w = np.asarray(inputs["out_w"], f)
    woT = _pmaj(np.ascontiguousarray(out_w.T) * WS).astype(f8)
    bo = np.ascontiguousarray(out_w @ bv + np.asarray(inputs["out_b"], f))

    w1 = np.asarray(inputs["ffn_w1"], f)
    n2w = np.asarray(inputs["norm2_w"], f)
    n2b = np.asarray(inputs["norm2_b"], f)
    w1T = _pmaj(np.ascontiguousarray((w1 * n2w[None, :]).T) * WS).astype(f8)
    b1v = w1 @ n2b + np.asarray(inputs["ffn_b1"], f)
    b1 = np.ascontiguousarray(b1v.reshape(NHID, P).T)
    w2T = _pmaj(np.ascontiguousarray(np.asarray(inputs["ffn_w2"], f).T) * WS).astype(f8)
    b2 = np.ascontiguousarray(np.asarray(inputs["ffn_b2"], f))

    ident = np.eye(P, dtype=bf)
    masks, masks0 = _host_masks()
    shared = dict(ident=ident, wqkvT=wqkvT, bq=bq, bk=bk, woT=woT, bo=bo,
                  w1T=w1T, b1=b1, w2T=w2T, b2=b2)
    in_maps = []
    for c in range(NCORES):
        b_, i = divmod(c, 4)
        own = x[b_, i * CH:(i + 1) * CH]
        if i == 0:
            halo = np.zeros((HALO, D), f)
        else:
            halo = x[b_, i * CH - HALO:i * CH]
        full = np.concatenate([halo, own], 0)
        xc = np.ascontiguousarray(
            full[_XIDX].reshape(NT, P, D).transpose(1, 0, 2))
        in_maps.append(dict(xc=xc, masks=(masks if i > 0 else masks0), **shared))
    return in_maps


def kernel(**inputs):
    global LAST_EXEC_NS, LAST_RESULTS
    from concourse.bass_utils import run_bass_kernel_spmd

    nc = get_nc()
    in_maps = make_in_maps(inputs)
    trace = bool(int(os.environ.get("BASS_KERNEL_TRACE", "0")))
    res = run_bass_kernel_spmd(nc, in_maps, core_ids=list(range(NCORES)),
                               trace=trace)
    LAST_EXEC_NS = res.exec_time_ns
    LAST_RESULTS = res
    out = np.zeros((B, L, D), np.float32)
    for c, r in enumerate(res.results):
        b_, i = divmod(c, 4)
        out[b_, i * CH + _TOKPERM] = r["y"]
    return out


# revision 6
# speedup vs baseline: 1.2226x; 1.2020x over previous
"""Trainium2 Bass kernel for a pre-norm transformer block with dilated
windowed causal attention (B=2, L=2048, D=512, H=8, DIL=2, WIN=256,
HIDDEN=2048).

Sharding: 8 cores = batch(2) x sequence-chunk(4 x 512 tokens). Each core
receives its 512-token chunk plus a 256-token halo (keys/values only) and
computes the full block for its tokens; no collectives.

Token axis is STREAM-MAJOR (dilation parity streams separated, host
reorders): t = [s0 own 256 | s1 own 256 | s0 halo 128 | s1 halo 128].
All attention slices are contiguous; host un-permutes the output.

Projection matmuls (QKV / out-proj / FFN) run in fp8e4 DoubleRow (two
128-deep k-planes per instruction). Weights are scaled x64 into fp8
range host-side; the descale is folded into the PSUM evacuation.
Attention S/PV matmuls stay bf16; S matmuls for the two heads of a
head-pair are emitted interleaved so they run on disjoint PE row groups
concurrently. The softmax denominator rides the PV matmul as a
ones-row; it is copied to SBUF, broadcast across partitions with a tiny
bf16 matmul (value 1/16), then inverted with reciprocal_approx_fast.
oT carries a x16 scale for fp8 range; out-proj descales by 1/(64*16).
FFN2 accumulation is interleaved into the FFN1 gelu pipeline.
"""
import os
import sys

os.environ.setdefault("MYCRO_LOCAL_CACHE", "1")
if "/opt/trn_rl_repo" not in sys.path:
    sys.path.insert(0, "/opt/trn_rl_repo")

import numpy as np

B, L, D, H, HD = 2, 2048, 512, 8, 64
HIDDEN = 4 * D
P = 128
CH = 512            # own tokens per core
HALO = 256
T = CH + HALO       # 768
NCORES = 8
EPS = 1e-5
SQ = 256            # own queries per parity stream
SCALE = 1.0 / 8.0   # 1/sqrt(HD)
WS = 64.0           # host-side fp8 weight scale
OS = 16.0           # oT fp8 scale (1/OS folded into esel)

NT = T // P         # 6
NO = CH // P        # 4
ND = D // P         # 4
NHID = HIDDEN // P  # 16

_nc = None
LAST_EXEC_NS = None
LAST_RESULTS = None

# key-tile -> t-range base: blk = s*3 + kt
KB = {(0, 0): 512, (0, 1): 0, (0, 2): 128,
      (1, 0): 640, (1, 1): 256, (1, 2): 384}


def _body(ctx, tc, I, y):
    import concourse.bass as bass  # noqa: F401
    from concourse import mybir

    nc = tc.nc
    f32 = mybir.dt.float32
    bf16 = mybir.dt.bfloat16
    f8 = mybir.dt.float8e4
    AF = mybir.ActivationFunctionType
    OP = mybir.AluOpType
    DR = mybir.MatmulPerfMode.DoubleRow

    consts = ctx.enter_context(tc.tile_pool(name="consts", bufs=1))
    big = ctx.enter_context(tc.tile_pool(name="big", bufs=1))
    work = ctx.enter_context(tc.tile_pool(name="work", bufs=4))
    pexp = ctx.enter_context(tc.tile_pool(name="pexp", bufs=8))

    mm = nc.tensor.matmul

    def bcast(ap, p=P):
        return bass.AP(tensor=ap.tensor, offset=ap.offset,
                       ap=[[0, p]] + [list(d) for d in ap.ap])

    # ---------- input DMAs (x first: critical path; weights behind) ----
    ident = consts.tile([P, P], bf16, tag="ident")
    nc.sync.dma_start(out=ident, in_=I["ident"])
    x_sb = big.tile([P, NT, D], f32, tag="x")
    for c0 in range(0, NT, 2):
        nc.sync.dma_start(out=x_sb[:, c0:c0 + 2, :], in_=I["xc"][:, c0:c0 + 2, :])
    masks_sb = consts.tile([P, 2, 2 * SQ], bf16, tag="masks")
    nc.sync.dma_start(out=masks_sb, in_=I["masks"])
    bq_sb = consts.tile([P, 4], f32, tag="bq")
    nc.sync.dma_start(out=bq_sb, in_=I["bq"])
    bk_sb = consts.tile([P, 4], f32, tag="bk")
    nc.sync.dma_start(out=bk_sb, in_=I["bk"])
    b1_sb = consts.tile([P, NHID], f32, tag="b1")
    nc.sync.dma_start(out=b1_sb, in_=I["b1"])
    wqkv_sb = big.tile([P, ND, 3 * D], f8, tag="wqkv")
    nc.sync.dma_start(out=wqkv_sb, in_=I["wqkvT"])
    wo_sb = big.tile([P, ND, D], f8, tag="wo")
    nc.sync.dma_start(out=wo_sb, in_=I["woT"])
    w1_sb = big.tile([P, ND, HIDDEN], f8, tag="w1")
    nc.sync.dma_start(out=w1_sb, in_=I["w1T"])
    w2_sb = big.tile([P, NHID, D], f8, tag="w2")
    nc.sync.dma_start(out=w2_sb, in_=I["w2T"])
    bo_sb = consts.tile([P, D], f32, tag="bo")
    nc.gpsimd.dma_start(out=bo_sb, in_=bcast(I["bo"]))
    b2_sb = consts.tile([P, D], f32, tag="b2")
    nc.gpsimd.dma_start(out=b2_sb, in_=bcast(I["b2"]))

    epst = consts.tile([P, 1], f32, tag="eps")
    nc.vector.memset(epst, EPS)
    esel = consts.tile([65, P], bf16, tag="esel")
    nc.vector.memset(esel, 0.0)
    nc.vector.memset(esel[0:1, 0:64], 1.0 / OS)
    nc.vector.memset(esel[64:65, 64:128], 1.0 / OS)

    # ---------- LN helper (stats+apply on Vector, sqrt on Scalar) ------
    def emit_ln(src, dst):
        st = work.tile([P, 6], f32, tag="bnst")
        nc.vector.bn_stats(st, src)
        mv = work.tile([P, 2], f32, tag="bnmv")
        nc.vector.bn_aggr(mv, st)
        r = work.tile([P, 1], f32, tag="lnr")
        nc.scalar.activation(r, mv[:, 1:2], AF.Sqrt, bias=epst, scale=1.0)
        r2 = work.tile([P, 1], f32, tag="lnr2")
        nc.vector.reciprocal(r2, r)
        nc.vector.tensor_scalar(out=dst, in0=src, scalar1=mv[:, 0:1],
                                scalar2=r2, op0=OP.subtract, op1=OP.mult)

    xhat = big.tile([P, NT, D], bf16, tag="xhat")
    xT = big.tile([P, ND, T], f8, tag="xT")
    kT = big.tile([P, 4, T], bf16, tag="kT")
    qT = big.tile([P, 4, CH], bf16, tag="qT")
    v_sb = big.tile([P, 6, H, 65], bf16, tag="v")

    with tc.tile_pool(name="pmm_h", bufs=2, space="PSUM") as pmm_h, \
         tc.tile_pool(name="ptp_h", bufs=2, space="PSUM") as ptp_h:

        # PE warm-up: keep the HAM activity window busy
        junk = pmm_h.tile([P, 512], f32, tag="ps")
        for _ in range(24):
            mm(junk[:, :P], ident, ident, start=True, stop=True)

        def emit_tp(cp):
            for dt_ in range(ND):
                pt = ptp_h.tile([P, 2 * P], bf16, tag="pt")
                for jj in range(2):
                    nc.tensor.transpose(pt[:, jj * P:(jj + 1) * P],
                                        xhat[:, 2 * cp + jj, dt_ * P:(dt_ + 1) * P],
                                        ident)
                if dt_ % 2 == 0:
                    nc.scalar.copy(xT[:, dt_, cp * 2 * P:(cp + 1) * 2 * P], pt)
                else:
                    nc.vector.tensor_copy(xT[:, dt_, cp * 2 * P:(cp + 1) * 2 * P], pt)

        def emit_k(t0, tn):
            for ot in range(4):
                ps = pmm_h.tile([P, 512], f32, tag="ps")
                for dp in range(2):
                    mm(ps[:, :tn], wqkv_sb[:, 2 * dp:2 * dp + 2, (4 + ot) * P:(5 + ot) * P],
                       xT[:, 2 * dp:2 * dp + 2, t0:t0 + tn],
                       start=(dp == 0), stop=(dp == 1), perf_mode=DR)
                if ot % 2 == 0:
                    nc.scalar.activation(kT[:, ot, t0:t0 + tn], ps[:, :tn],
                                         AF.Identity, bias=bk_sb[:, ot:ot + 1],
                                         scale=1.0 / WS)
                else:
                    nc.vector.tensor_scalar(out=kT[:, ot, t0:t0 + tn],
                                            in0=ps[:, :tn], scalar1=1.0 / WS,
                                            scalar2=bk_sb[:, ot:ot + 1],
                                            op0=OP.mult, op1=OP.add)

        def emit_q():
            for ot in range(4):
                ps = pmm_h.tile([P, 512], f32, tag="ps")
                for dp in range(2):
                    mm(ps, wqkv_sb[:, 2 * dp:2 * dp + 2, ot * P:(ot + 1) * P],
                       xT[:, 2 * dp:2 * dp + 2, 0:CH],
                       start=(dp == 0), stop=(dp == 1), perf_mode=DR)
                nc.scalar.activation(qT[:, ot, :], ps, AF.Identity,
                                     bias=bq_sb[:, ot:ot + 1], scale=1.0 / WS)

        def emit_v(s, kt):
            t0 = KB[(s, kt)]
            blk = s * 3 + kt
            ps = pmm_h.tile([P, 512], f32, tag="ps")
            for dp in range(2):
                mm(ps, xT[:, 2 * dp:2 * dp + 2, t0:t0 + P],
                   wqkv_sb[:, 2 * dp:2 * dp + 2, 2 * D:3 * D],
                   start=(dp == 0), stop=(dp == 1), perf_mode=DR)
            nc.vector.tensor_scalar(out=v_sb[:, blk, :, 0:64],
                                    in0=ps.rearrange("p (h c) -> p h c", h=H),
                                    scalar1=1.0 / WS, scalar2=0.0,
                                    op0=OP.mult, op1=OP.add)

        for i in range(6):
            nc.vector.memset(v_sb[:, i, :, 64:65], 1.0)

        emit_ln(x_sb[:, 0, :], xhat[:, 0, :])
        emit_ln(x_sb[:, 1, :], xhat[:, 1, :])
        emit_tp(0)
        emit_ln(x_sb[:, 2, :], xhat[:, 2, :])
        emit_ln(x_sb[:, 3, :], xhat[:, 3, :])
        emit_tp(1)
        emit_q()
        emit_k(0, 512)
        emit_v(0, 1)
        emit_v(0, 2)
        emit_ln(x_sb[:, 4, :], xhat[:, 4, :])
        emit_ln(x_sb[:, 5, :], xhat[:, 5, :])
        emit_tp(2)
        emit_k(512, 256)
        emit_v(1, 1)
        emit_v(1, 2)
        emit_v(0, 0)
        emit_v(1, 0)

        # out-proj bias pre-add (after LN reads of x_sb; used by out-proj)
        for tt in range(NO):
            nc.gpsimd.tensor_add(x_sb[:, tt, :], x_sb[:, tt, :], bo_sb)

    # ---------- attention ----------
    # masks_sb[:, 0] = [tril|tril|triu|triu]  (kt0 s0,s1 | kt2 s0,s1)
    # masks_sb[:, 1] = [triu|tril|triu|tril]  (kt1: s0 qb0,qb1 | s1 qb0,qb1)
    oU = big.tile([P, 4, CH], bf16, tag="oU")
    oT = big.tile([P, 4, CH], f8, tag="oT")
    den4s = {}
    for hp in range(4):
        den = work.tile([65, CH], bf16, tag="den")
        den4s[hp] = den
        nc.vector.memset(den, 1.0)

    with tc.tile_pool(name="pa_s", bufs=4, space="PSUM") as pa_s, \
         tc.tile_pool(name="pa_o", bufs=3, space="PSUM") as pa_o:

        def emit_S02(hp, alt):
            ps = {hh: pa_s.tile([P, 2 * SQ], f32, tag="ps_s", name=f"ps02_{hp}_{hh}")
                  for hh in (0, 1)}
            for ri, (s, kt) in enumerate([(0, 0), (1, 0), (0, 2), (1, 2)]):
                k0 = KB[(s, kt)]
                q0 = s * SQ if kt == 0 else s * SQ + P
                for hh in (0, 1):
                    lo = hh * 64
                    mm(ps[hh][:, ri * P:(ri + 1) * P],
                       kT[lo:lo + 64, hp, k0:k0 + P],
                       qT[lo:lo + 64, hp, q0:q0 + P],
                       start=True, stop=True)
            out = {}
            for hh in (0, 1):
                p_sb = pexp.tile([P, 2 * SQ], bf16, tag="p_sb")
                nc.scalar.activation(p_sb, ps[hh], AF.Exp, scale=SCALE)
                if (hh == 0) == alt:
                    nc.gpsimd.tensor_mul(p_sb, p_sb, masks_sb[:, 0, :])
                else:
                    nc.vector.tensor_mul(p_sb, p_sb, masks_sb[:, 0, :])
                out[hh] = p_sb
            return out

        def emit_S1(hp, alt):
            ps = {hh: pa_s.tile([P, 2 * SQ], f32, tag="ps_s", name=f"ps1_{hp}_{hh}")
                  for hh in (0, 1)}
            for s in (0, 1):
                k0 = KB[(s, 1)]
                q0 = s * SQ
                for hh in (0, 1):
                    lo = hh * 64
                    mm(ps[hh][:, s * SQ:(s + 1) * SQ],
                       kT[lo:lo + 64, hp, k0:k0 + P],
                       qT[lo:lo + 64, hp, q0:q0 + SQ],
                       start=True, stop=True)
            out = {}
            for hh in (0, 1):
                p_sb = pexp.tile([P, 2 * SQ], bf16, tag="p_sb")
                nc.scalar.activation(p_sb, ps[hh], AF.Exp, scale=SCALE)
                if (hh == 0) != alt:
                    nc.gpsimd.tensor_mul(p_sb, p_sb, masks_sb[:, 1, :])
                else:
                    nc.vector.tensor_mul(p_sb, p_sb, masks_sb[:, 1, :])
                out[hh] = p_sb
            return out

        def emit_PV(hp, hh, p02, p1):
            h = 2 * hp + hh
            lo = hh * 64
            po = pa_o.tile([P, 2 * SQ], f32, tag="po")
            for s in range(2):
                qa = s * SQ
                qb = s * SQ + P
                mm(po[:65, qa:qa + P], v_sb[:, s * 3 + 0, h, :],
                   p02[:, s * P:(s + 1) * P], start=True, stop=False)
                mm(po[:65, qa:qa + P], v_sb[:, s * 3 + 1, h, :],
                   p1[:, s * 2 * P:s * 2 * P + P], start=False, stop=True)
                mm(po[:65, qb:qb + P], v_sb[:, s * 3 + 1, h, :],
                   p1[:, s * 2 * P + P:(s + 1) * 2 * P], start=True, stop=False)
                mm(po[:65, qb:qb + P], v_sb[:, s * 3 + 2, h, :],
                   p02[:, 2 * SQ // 2 + s * P:2 * SQ // 2 + (s + 1) * P],
                   start=False, stop=True)
            if hh == 0:
                nc.scalar.copy(oU[lo:lo + 64, hp, :], po[:64, :])
            else:
                nc.vector.tensor_copy(oU[lo:lo + 64, hp, :], po[:64, :])
            with nc.allow_low_precision("softmax denominator in bf16"):
                nc.vector.tensor_copy(den4s[hp][lo:lo + 1, :], po[64:65, :])

        def emit_norm(hp):
            pb = pa_o.tile([P, 2 * SQ], f32, tag="po")
            mm(pb, esel, den4s[hp], start=True, stop=True)
            rb = work.tile([P, CH], f32, tag="rb")
            nc.vector.reciprocal_approx_fast(rb, pb)
            nc.vector.tensor_mul(oT[:, hp, :], oU[:, hp, :], rb)

        prev = None
        for hp in range(4):
            alt = hp % 2 == 0
            p02 = emit_S02(hp, alt)
            p1 = emit_S1(hp, alt)
            if prev is not None:
                php, p02p, p1p = prev
                emit_PV(php, 0, p02p[0], p1p[0])
                emit_PV(php, 1, p02p[1], p1p[1])
                emit_norm(php)
            prev = (hp, p02, p1)
        php, p02p, p1p = prev
        emit_PV(php, 0, p02p[0], p1p[0])
        emit_PV(php, 1, p02p[1], p1p[1])
        emit_norm(php)

    # ---------- tail: out-proj, LN2, x2T, FFN1+FFN2 interleaved --------
    res1 = big.tile([P, NO, D], f32, tag="res1")
    xhat2 = big.tile([P, NO, D], bf16, tag="xhat2")
    x2T = big.tile([P, ND, CH], f8, tag="x2T")
    g_sb = big.tile([P, NHID, CH], f8, tag="g")
    fin = big.tile([P, NO, D], f32, tag="fin")

    with tc.tile_pool(name="pmm_t", bufs=2, space="PSUM") as pmm_t, \
         tc.tile_pool(name="ptp_t", bufs=2, space="PSUM") as ptp_t, \
         tc.tile_pool(name="pffn", bufs=1, space="PSUM") as pffn:

        for tt in range(NO):
            ps = pmm_t.tile([P, 512], f32, tag="ps")
            for pp in range(2):
                mm(ps, oT[:, 2 * pp:2 * pp + 2, tt * P:(tt + 1) * P],
                   wo_sb[:, 2 * pp:2 * pp + 2, :],
                   start=(pp == 0), stop=(pp == 1), perf_mode=DR)
            nc.vector.scalar_tensor_tensor(out=res1[:, tt, :], in0=ps,
                                           scalar=1.0 / (WS * OS),
                                           in1=x_sb[:, tt, :],
                                           op0=OP.mult, op1=OP.add)
            emit_ln(res1[:, tt, :], xhat2[:, tt, :])
            if tt % 2 == 1:
                cp = tt // 2
                for dt_ in range(ND):
                    pt = ptp_t.tile([P, 2 * P], bf16, tag="pt")
                    for jj in range(2):
                        nc.tensor.transpose(pt[:, jj * P:(jj + 1) * P],
                                            xhat2[:, 2 * cp + jj, dt_ * P:(dt_ + 1) * P],
                                            ident)
                    if dt_ % 2 == 0:
                        nc.scalar.copy(x2T[:, dt_, cp * 2 * P:(cp + 1) * 2 * P], pt)
                    else:
                        nc.vector.tensor_copy(x2T[:, dt_, cp * 2 * P:(cp + 1) * 2 * P], pt)

        for tt in range(NO):
            nc.vector.tensor_add(res1[:, tt, :], res1[:, tt, :], b2_sb)

        ps_tt = [pffn.tile([P, 512], f32, tag=f"pf{tt}", name=f"pf{tt}") for tt in range(NO)]
        for ht in range(NHID):
            ps = pmm_t.tile([P, 512], f32, tag="ps")
            for dp in range(2):
                mm(ps, w1_sb[:, 2 * dp:2 * dp + 2, ht * P:(ht + 1) * P],
                   x2T[:, 2 * dp:2 * dp + 2, :],
                   start=(dp == 0), stop=(dp == 1), perf_mode=DR)
            nc.scalar.activation(g_sb[:, ht, :], ps, AF.Gelu,
                                 bias=b1_sb[:, ht:ht + 1], scale=1.0 / WS)
            if ht % 2 == 1:
                hq = ht // 2
                for tt in range(NO):
                    mm(ps_tt[tt], g_sb[:, ht - 1:ht + 1, tt * P:(tt + 1) * P],
                       w2_sb[:, ht - 1:ht + 1, :],
                       start=(hq == 0), stop=(hq == NHID // 2 - 1), perf_mode=DR)
        for tt in range(NO):
            nc.vector.scalar_tensor_tensor(out=fin[:, tt, :], in0=ps_tt[tt],
                                           scalar=1.0 / WS, in1=res1[:, tt, :],
                                           op0=OP.mult, op1=OP.add)

        yr = y.rearrange("(j p) d -> p j d", p=P)
        for tt in range(NO):
            nc.sync.dma_start(out=yr[:, tt, :], in_=fin[:, tt, :])


def _build():
    from contextlib import ExitStack

    import concourse.bacc as bacc
    import concourse.tile as tile
    from concourse import mybir

    f32 = mybir.dt.float32
    bf16 = mybir.dt.bfloat16
    f8 = mybir.dt.float8e4
    nc = bacc.Bacc("TRN2", target_bir_lowering=False, debug=False,
                   enable_asserts=False, num_devices=NCORES)
    I = {}

    def inp(name, shape, dt_):
        I[name] = nc.dram_tensor(name, list(shape), dt_, kind="ExternalInput").ap()

    inp("xc", (P, NT, D), f32)
    inp("ident", (P, P), bf16)
    inp("wqkvT", (P, ND, 3 * D), f8)
    inp("bq", (P, 4), f32)
    inp("bk", (P, 4), f32)
    inp("woT", (P, ND, D), f8)
    inp("bo", (D,), f32)
    inp("w1T", (P, ND, HIDDEN), f8)
    inp("b1", (P, NHID), f32)
    inp("w2T", (P, NHID, D), f8)
    inp("b2", (D,), f32)
    inp("masks", (P, 2, 2 * SQ), bf16)
    y = nc.dram_tensor("y", [CH, D], f32, kind="ExternalOutput").ap()

    with tile.TileContext(nc) as tc:
        with ExitStack() as ctx:
            _body(ctx, tc, I, y)
    nc.compile()
    return nc


def _host_masks():
    import ml_dtypes
    tril = np.tril(np.ones((P, P), np.float32))
    triu = np.triu(np.ones((P, P), np.float32))
    kt02 = np.concatenate([tril, tril, triu, triu], 1)
    kt1 = np.concatenate([triu, tril, triu, tril], 1)
    m = np.stack([kt02, kt1]).astype(ml_dtypes.bfloat16)
    m0 = m.copy()
    m0[0, :, 0:256] = 0.0  # first chunk of each batch: halo keys invalid
    m = np.ascontiguousarray(m.transpose(1, 0, 2))
    m0 = np.ascontiguousarray(m0.transpose(1, 0, 2))
    return m, m0


def get_nc():
    global _nc
    if _nc is None:
        _nc = _build()
    return _nc


def _pmaj(a, p=P):
    """[N*p, F...] row-major -> [p, N, F...] partition-major contiguous."""
    n = a.shape[0] // p
    return np.ascontiguousarray(
        a.reshape((n, p) + a.shape[1:]).transpose((1, 0) + tuple(range(2, a.ndim + 1))))


# stream-major chunk gather: full[768] = [halo 256 | own 512]
_a = np.arange(128)
_XIDX = np.concatenate([256 + 2 * _a, 512 + 2 * _a, 257 + 2 * _a,
                        513 + 2 * _a, 2 * _a, 1 + 2 * _a])
# y rows (chunk-major stream order) -> original own-token index
_TOKPERM = np.concatenate([2 * _a, 256 + 2 * _a, 1 + 2 * _a, 257 + 2 * _a])


def make_in_maps(inputs):
    import ml_dtypes
    f = np.float32
    bf = ml_dtypes.bfloat16
    f8 = ml_dtypes.float8_e4m3
    x = np.asarray(inputs["x"], f)
    qkv_w = np.asarray(inputs["qkv_w"], f)
    n1w = np.asarray(inputs["norm1_w"], f)
    n1b = np.asarray(inputs["norm1_b"], f)
    wqkv_f = qkv_w * n1w[None, :]
    bqkv = qkv_w @ n1b + np.asarray(inputs["qkv_b"], f)
    wqkvT = _pmaj(np.ascontiguousarray(wqkv_f.T) * WS).astype(f8)
    bq = np.ascontiguousarray(bqkv[0:D].reshape(4, P).T)
    bk = np.ascontiguousarray(bqkv[D:2 * D].reshape(4, P).T)
    bv = np.ascontiguousarray(bqkv[2 * D:3 * D])

    out_w = np.asarray(inputs["out_w"], f)
    woT = _pmaj(np.ascontiguousarray(out_w.T) * WS).astype(f8)
    bo = np.ascontiguousarray(out_w @ bv + np.asarray(inputs["out_b"], f))

    w1 = np.asarray(inputs["ffn_w1"], f)
    n2w = np.asarray(inputs["norm2_w"], f)
    n2b = np.asarray(inputs["norm2_b"], f)
    w1T = _pmaj(np.ascontiguousarray((w1 * n2w[None, :]).T) * WS).astype(f8)
    b1v = w1 @ n2b + np.asarray(inputs["ffn_b1"], f)
    b1 = np.ascontiguousarray(b1v.reshape(NHID, P).T)
    w2T = _pmaj(np.ascontiguousarray(np.asarray(inputs["ffn_w2"], f).T) * WS).astype(f8)
    b2 = np.ascontiguousarray(np.asarray(inputs["ffn_b2"], f))

    ident = np.eye(P, dtype=bf)
    masks, masks0 = _host_masks()
    shared = dict(ident=ident, wqkvT=wqkvT, bq=bq, bk=bk, woT=woT, bo=bo,
                  w1T=w1T, b1=b1, w2T=w2T, b2=b2)
    in_maps = []
    for c in range(NCORES):
        b_, i = divmod(c, 4)
        own = x[b_, i * CH:(i + 1) * CH]
        if i == 0:
            halo = np.zeros((HALO, D), f)
        else:
            halo = x[b_, i * CH - HALO:i * CH]
        full = np.concatenate([halo, own], 0)
        xc = np.ascontiguousarray(
            full[_XIDX].reshape(NT, P, D).transpose(1, 0, 2))
        in_maps.append(dict(xc=xc, masks=(masks if i > 0 else masks0), **shared))
    return in_maps


def kernel(**inputs):
    global LAST_EXEC_NS, LAST_RESULTS
    from concourse.bass_utils import run_bass_kernel_spmd

    nc = get_nc()
    in_maps = make_in_maps(inputs)
    trace = bool(int(os.environ.get("BASS_KERNEL_TRACE", "0")))
    res = run_bass_kernel_spmd(nc, in_maps, core_ids=list(range(NCORES)),
                               trace=trace)
    LAST_EXEC_NS = res.exec_time_ns
    LAST_RESULTS = res
    out = np.zeros((B, L, D), np.float32)
    for c, r in enumerate(res.results):
        b_, i = divmod(c, 4)
        out[b_, i * CH + _TOKPERM] = r["y"]
    return out


# revision 8
# speedup vs baseline: 1.3068x; 1.0689x over previous
"""Trainium2 Bass kernel for a pre-norm transformer block with dilated
windowed causal attention (B=2, L=2048, D=512, H=8, DIL=2, WIN=256,
HIDDEN=2048).

Sharding: 8 cores = batch(2) x sequence-chunk(4 x 512 tokens). Each core
receives its 512-token chunk plus a 256-token halo (keys/values only) and
computes the full block for its tokens; no collectives.

Token axis is STREAM-MAJOR (dilation parity streams separated, host
reorders): t = [s0 own 256 | s1 own 256 | s0 halo 128 | s1 halo 128].
All attention slices are contiguous; host un-permutes the output.

Projection matmuls (QKV / out-proj / FFN) run in fp8e4 DoubleRow (two
128-deep k-planes per instruction). Weights are scaled x64 into fp8
range host-side; the descale is folded into the PSUM evacuation.
Attention S/PV matmuls stay bf16; S matmuls for the two heads of a
head-pair are emitted interleaved so they run on disjoint PE row groups
concurrently. The softmax denominator rides the PV matmul as a
ones-row; it is copied to SBUF, broadcast across partitions with a tiny
bf16 matmul (value 1/16), then inverted with reciprocal_approx_fast.
oT carries a x16 scale for fp8 range; out-proj descales by 1/(64*16).
FFN2 accumulation is interleaved into the FFN1 gelu pipeline.
"""
import os
import sys

os.environ.setdefault("MYCRO_LOCAL_CACHE", "1")
if "/opt/trn_rl_repo" not in sys.path:
    sys.path.insert(0, "/opt/trn_rl_repo")

import numpy as np

B, L, D, H, HD = 2, 2048, 512, 8, 64
HIDDEN = 4 * D
P = 128
CH = 512            # own tokens per core
HALO = 256
T = CH + HALO       # 768
NCORES = 8
EPS = 1e-5
SQ = 256            # own queries per parity stream
SCALE = 1.0 / 8.0   # 1/sqrt(HD)
WS = 64.0           # host-side fp8 weight scale
OS = 16.0           # oT fp8 scale (1/OS folded into esel)

NT = T // P         # 6
NO = CH // P        # 4
ND = D // P         # 4
NHID = HIDDEN // P  # 16

_nc = None
LAST_EXEC_NS = None
LAST_RESULTS = None

# key-tile -> t-range base: blk = s*3 + kt
KB = {(0, 0): 512, (0, 1): 0, (0, 2): 128,
      (1, 0): 640, (1, 1): 256, (1, 2): 384}


def _body(ctx, tc, I, y):
    import concourse.bass as bass  # noqa: F401
    from concourse import mybir

    nc = tc.nc
    f32 = mybir.dt.float32
    bf16 = mybir.dt.bfloat16
    f8 = mybir.dt.float8e4
    AF = mybir.ActivationFunctionType
    OP = mybir.AluOpType
    DR = mybir.MatmulPerfMode.DoubleRow

    consts = ctx.enter_context(tc.tile_pool(name="consts", bufs=1))
    big = ctx.enter_context(tc.tile_pool(name="big", bufs=1))
    work = ctx.enter_context(tc.tile_pool(name="work", bufs=4))
    pexp = ctx.enter_context(tc.tile_pool(name="pexp", bufs=8))

    mm = nc.tensor.matmul

    def bcast(ap, p=P):
        return bass.AP(tensor=ap.tensor, offset=ap.offset,
                       ap=[[0, p]] + [list(d) for d in ap.ap])

    # ---------- input DMAs (x first: critical path; weights behind) ----
    from concourse.tile import add_dep_helper

    ident = consts.tile([P, P], bf16, tag="ident")
    nc.sync.dma_start(out=ident, in_=I["ident"])
    x_sb = big.tile([P, NT, D], f32, tag="x")
    xdma = None
    for c0 in range(0, NT, 2):
        xdma = nc.sync.dma_start(out=x_sb[:, c0:c0 + 2, :],
                                 in_=I["xc"][:, c0:c0 + 2, :])
    masks_sb = consts.tile([P, 2, 2 * SQ], bf16, tag="masks")
    nc.sync.dma_start(out=masks_sb, in_=I["masks"])
    bq_sb = consts.tile([P, 4], f32, tag="bq")
    nc.sync.dma_start(out=bq_sb, in_=I["bq"])
    bk_sb = consts.tile([P, 4], f32, tag="bk")
    nc.sync.dma_start(out=bk_sb, in_=I["bk"])
    b1_sb = consts.tile([P, NHID], f32, tag="b1")
    nc.sync.dma_start(out=b1_sb, in_=I["b1"])
    # weights share SDMA bandwidth with x if launched together; make them
    # wait for the last x chunk so LN1 starts ~10us earlier.
    wqkv_sb = big.tile([P, ND, 3 * D], f8, tag="wqkv")
    wd = nc.sync.dma_start(out=wqkv_sb, in_=I["wqkvT"])
    add_dep_helper(wd.ins, xdma.ins, reason="stagger weight DMA behind x")
    wo_sb = big.tile([P, ND, D], f8, tag="wo")
    wd = nc.sync.dma_start(out=wo_sb, in_=I["woT"])
    add_dep_helper(wd.ins, xdma.ins, reason="stagger weight DMA behind x")
    w1_sb = big.tile([P, ND, HIDDEN], f8, tag="w1")
    wd = nc.sync.dma_start(out=w1_sb, in_=I["w1T"])
    add_dep_helper(wd.ins, xdma.ins, reason="stagger weight DMA behind x")
    w2_sb = big.tile([P, NHID, D], f8, tag="w2")
    wd = nc.sync.dma_start(out=w2_sb, in_=I["w2T"])
    add_dep_helper(wd.ins, xdma.ins, reason="stagger weight DMA behind x")
    bo_sb = consts.tile([P, D], f32, tag="bo")
    nc.gpsimd.dma_start(out=bo_sb, in_=bcast(I["bo"]))
    b2_sb = consts.tile([P, D], f32, tag="b2")
    nc.gpsimd.dma_start(out=b2_sb, in_=bcast(I["b2"]))

    epst = consts.tile([P, 1], f32, tag="eps")
    nc.vector.memset(epst, EPS)
    esel = consts.tile([P, P], bf16, tag="esel")
    nc.gpsimd.memset(esel, 0.0)
    nc.gpsimd.memset(esel[0:1, 0:64], 1.0 / OS)
    nc.gpsimd.memset(esel[64:65, 64:128], 1.0 / OS)

    # ---------- LN helper (stats+apply on Vector, sqrt on Scalar) ------
    def emit_ln(src, dst):
        st = work.tile([P, 6], f32, tag="bnst")
        nc.vector.bn_stats(st, src)
        mv = work.tile([P, 2], f32, tag="bnmv")
        nc.vector.bn_aggr(mv, st)
        r = work.tile([P, 1], f32, tag="lnr")
        nc.scalar.activation(r, mv[:, 1:2], AF.Sqrt, bias=epst, scale=1.0)
        r2 = work.tile([P, 1], f32, tag="lnr2")
        nc.vector.reciprocal(r2, r)
        nc.vector.tensor_scalar(out=dst, in0=src, scalar1=mv[:, 0:1],
                                scalar2=r2, op0=OP.subtract, op1=OP.mult)

    xhat = big.tile([P, NT, D], bf16, tag="xhat")
    xT = big.tile([P, ND, T], f8, tag="xT")
    kT = big.tile([P, 4, T], bf16, tag="kT")
    qT = big.tile([P, 4, CH], bf16, tag="qT")
    v_sb = big.tile([P, 6, H, 65], bf16, tag="v")

    with tc.tile_pool(name="pmm_h", bufs=2, space="PSUM") as pmm_h, \
         tc.tile_pool(name="ptp_h", bufs=2, space="PSUM") as ptp_h:

        # PE warm-up: keep the HAM activity window busy
        junk = pmm_h.tile([P, 512], f32, tag="ps")
        for _ in range(24):
            mm(junk[:, :P], ident, ident, start=True, stop=True)

        def emit_tp(cp):
            for dt_ in range(ND):
                pt = ptp_h.tile([P, 2 * P], bf16, tag="pt")
                for jj in range(2):
                    nc.tensor.transpose(pt[:, jj * P:(jj + 1) * P],
                                        xhat[:, 2 * cp + jj, dt_ * P:(dt_ + 1) * P],
                                        ident)
                if dt_ % 2 == 0:
                    nc.scalar.copy(xT[:, dt_, cp * 2 * P:(cp + 1) * 2 * P], pt)
                else:
                    nc.vector.tensor_copy(xT[:, dt_, cp * 2 * P:(cp + 1) * 2 * P], pt)

        def emit_k(t0, tn):
            for ot in range(4):
                ps = pmm_h.tile([P, 512], f32, tag="ps")
                for dp in range(2):
                    mm(ps[:, :tn], wqkv_sb[:, 2 * dp:2 * dp + 2, (4 + ot) * P:(5 + ot) * P],
                       xT[:, 2 * dp:2 * dp + 2, t0:t0 + tn],
                       start=(dp == 0), stop=(dp == 1), perf_mode=DR)
                if ot % 2 == 0:
                    nc.scalar.activation(kT[:, ot, t0:t0 + tn], ps[:, :tn],
                                         AF.Identity, bias=bk_sb[:, ot:ot + 1],
                                         scale=1.0 / WS)
                else:
                    nc.vector.tensor_scalar(out=kT[:, ot, t0:t0 + tn],
                                            in0=ps[:, :tn], scalar1=1.0 / WS,
                                            scalar2=bk_sb[:, ot:ot + 1],
                                            op0=OP.mult, op1=OP.add)

        def emit_q():
            for ot in range(4):
                ps = pmm_h.tile([P, 512], f32, tag="ps")
                for dp in range(2):
                    mm(ps, wqkv_sb[:, 2 * dp:2 * dp + 2, ot * P:(ot + 1) * P],
                       xT[:, 2 * dp:2 * dp + 2, 0:CH],
                       start=(dp == 0), stop=(dp == 1), perf_mode=DR)
                nc.scalar.activation(qT[:, ot, :], ps, AF.Identity,
                                     bias=bq_sb[:, ot:ot + 1], scale=1.0 / WS)

        def emit_v(s, kt):
            t0 = KB[(s, kt)]
            blk = s * 3 + kt
            ps = pmm_h.tile([P, 512], f32, tag="ps")
            for dp in range(2):
                mm(ps, xT[:, 2 * dp:2 * dp + 2, t0:t0 + P],
                   wqkv_sb[:, 2 * dp:2 * dp + 2, 2 * D:3 * D],
                   start=(dp == 0), stop=(dp == 1), perf_mode=DR)
            nc.vector.tensor_scalar(out=v_sb[:, blk, :, 0:64],
                                    in0=ps.rearrange("p (h c) -> p h c", h=H),
                                    scalar1=1.0 / WS, scalar2=0.0,
                                    op0=OP.mult, op1=OP.add)

        nc.gpsimd.memset(v_sb[:, :, :, 64:65], 1.0)

        emit_ln(x_sb[:, 0, :], xhat[:, 0, :])
        emit_ln(x_sb[:, 1, :], xhat[:, 1, :])
        emit_tp(0)
        emit_ln(x_sb[:, 2, :], xhat[:, 2, :])
        emit_ln(x_sb[:, 3, :], xhat[:, 3, :])
        emit_tp(1)
        emit_q()
        emit_k(0, 512)
        emit_v(0, 1)
        emit_v(0, 2)
        emit_ln(x_sb[:, 4, :], xhat[:, 4, :])
        emit_ln(x_sb[:, 5, :], xhat[:, 5, :])
        emit_tp(2)
        emit_k(512, 256)
        emit_v(1, 1)
        emit_v(1, 2)
        emit_v(0, 0)
        emit_v(1, 0)

        # out-proj bias pre-add (after LN reads of x_sb; used by out-proj)
        for tt in range(NO):
            nc.gpsimd.tensor_add(x_sb[:, tt, :], x_sb[:, tt, :], bo_sb)

    # ---------- attention ----------
    # masks_sb[:, 0] = [tril|tril|triu|triu]  (kt0 s0,s1 | kt2 s0,s1)
    # masks_sb[:, 1] = [triu|tril|triu|tril]  (kt1: s0 qb0,qb1 | s1 qb0,qb1)
    oU = big.tile([P, 4, CH], bf16, tag="oU")
    oT = big.tile([P, 4, CH], f8, tag="oT")
    den4s = {}
    for hp in range(4):
        den = work.tile([P, CH], bf16, tag="den")
        den4s[hp] = den
        nc.gpsimd.memset(den, 1.0)

    with tc.tile_pool(name="pa_s", bufs=4, space="PSUM") as pa_s, \
         tc.tile_pool(name="pa_o", bufs=3, space="PSUM") as pa_o:

        def emit_S02(hp, alt):
            ps = {hh: pa_s.tile([P, 2 * SQ], f32, tag="ps_s", name=f"ps02_{hp}_{hh}")
                  for hh in (0, 1)}
            for hh in (0, 1):
                mm(ps[hh], ident, masks_sb[:, 0, :], start=True, stop=False)
            for ri, (s, kt) in enumerate([(0, 0), (1, 0), (0, 2), (1, 2)]):
                k0 = KB[(s, kt)]
                q0 = s * SQ if kt == 0 else s * SQ + P
                for hh in (0, 1):
                    lo = hh * 64
                    mm(ps[hh][:, ri * P:(ri + 1) * P],
                       kT[lo:lo + 64, hp, k0:k0 + P],
                       qT[lo:lo + 64, hp, q0:q0 + P],
                       start=False, stop=(ri == 3))
            out = {}
            for hh in (0, 1):
                p_sb = pexp.tile([P, 2 * SQ], bf16, tag="p_sb")
                nc.scalar.activation(p_sb, ps[hh], AF.Exp, scale=SCALE)
                out[hh] = p_sb
            return out

        def emit_S1(hp, alt):
            ps = {hh: pa_s.tile([P, 2 * SQ], f32, tag="ps_s", name=f"ps1_{hp}_{hh}")
                  for hh in (0, 1)}
            for hh in (0, 1):
                mm(ps[hh], ident, masks_sb[:, 1, :], start=True, stop=False)
            for s in (0, 1):
                k0 = KB[(s, 1)]
                q0 = s * SQ
                for hh in (0, 1):
                    lo = hh * 64
                    mm(ps[hh][:, s * SQ:(s + 1) * SQ],
                       kT[lo:lo + 64, hp, k0:k0 + P],
                       qT[lo:lo + 64, hp, q0:q0 + SQ],
                       start=False, stop=(s == 1))
            out = {}
            for hh in (0, 1):
                p_sb = pexp.tile([P, 2 * SQ], bf16, tag="p_sb")
                nc.scalar.activation(p_sb, ps[hh], AF.Exp, scale=SCALE)
                out[hh] = p_sb
            return out

        def emit_PV(hp, hh, p02, p1):
            h = 2 * hp + hh
            lo = hh * 64
            po = pa_o.tile([P, 2 * SQ], f32, tag="po")
            for s in range(2):
                qa = s * SQ
                qb = s * SQ + P
                mm(po[:65, qa:qa + P], v_sb[:, s * 3 + 0, h, :],
                   p02[:, s * P:(s + 1) * P], start=True, stop=False)
                mm(po[:65, qa:qa + P], v_sb[:, s * 3 + 1, h, :],
                   p1[:, s * 2 * P:s * 2 * P + P], start=False, stop=True)
                mm(po[:65, qb:qb + P], v_sb[:, s * 3 + 1, h, :],
                   p1[:, s * 2 * P + P:(s + 1) * 2 * P], start=True, stop=False)
                mm(po[:65, qb:qb + P], v_sb[:, s * 3 + 2, h, :],
                   p02[:, 2 * SQ // 2 + s * P:2 * SQ // 2 + (s + 1) * P],
                   start=False, stop=True)
            if hh == 0:
                nc.scalar.copy(oU[lo:lo + 64, hp, :], po[:64, :])
            else:
                nc.vector.tensor_copy(oU[lo:lo + 64, hp, :], po[:64, :])
            nc.scalar.copy(den4s[hp][lo:lo + 1, :], po[64:65, :])

        def emit_norm(hp):
            pb = pa_o.tile([P, 2 * SQ], f32, tag="po")
            mm(pb, esel, den4s[hp], start=True, stop=True)
            rb = work.tile([P, CH], f32, tag="rb")
            nc.vector.reciprocal_approx_fast(rb, pb)
            nc.vector.tensor_mul(oT[:, hp, :], oU[:, hp, :], rb)

        prev = None
        for hp in range(4):
            alt = hp % 2 == 0
            p02 = emit_S02(hp, alt)
            p1 = emit_S1(hp, alt)
            if prev is not None:
                php, p02p, p1p = prev
                emit_PV(php, 0, p02p[0], p1p[0])
                emit_PV(php, 1, p02p[1], p1p[1])
                emit_norm(php)
            prev = (hp, p02, p1)
        php, p02p, p1p = prev
        emit_PV(php, 0, p02p[0], p1p[0])
        emit_PV(php, 1, p02p[1], p1p[1])
        emit_norm(php)

    # ---------- tail: out-proj, LN2, x2T, FFN1+FFN2 interleaved --------
    res1 = big.tile([P, NO, D], f32, tag="res1")
    xhat2 = big.tile([P, NO, D], bf16, tag="xhat2")
    x2T = big.tile([P, ND, CH], f8, tag="x2T")
    g_sb = big.tile([P, NHID, CH], f8, tag="g")
    fin = big.tile([P, NO, D], f32, tag="fin")

    with tc.tile_pool(name="pmm_t", bufs=2, space="PSUM") as pmm_t, \
         tc.tile_pool(name="ptp_t", bufs=2, space="PSUM") as ptp_t, \
         tc.tile_pool(name="pffn", bufs=1, space="PSUM") as pffn:

        for tt in range(NO):
            ps = pmm_t.tile([P, 512], f32, tag="ps")
            for pp in range(2):
                mm(ps, oT[:, 2 * pp:2 * pp + 2, tt * P:(tt + 1) * P],
                   wo_sb[:, 2 * pp:2 * pp + 2, :],
                   start=(pp == 0), stop=(pp == 1), perf_mode=DR)
            nc.vector.scalar_tensor_tensor(out=res1[:, tt, :], in0=ps,
                                           scalar=1.0 / (WS * OS),
                                           in1=x_sb[:, tt, :],
                                           op0=OP.mult, op1=OP.add)
            emit_ln(res1[:, tt, :], xhat2[:, tt, :])
            if tt % 2 == 1:
                cp = tt // 2
                for dt_ in range(ND):
                    pt = ptp_t.tile([P, 2 * P], bf16, tag="pt")
                    for jj in range(2):
                        nc.tensor.transpose(pt[:, jj * P:(jj + 1) * P],
                                            xhat2[:, 2 * cp + jj, dt_ * P:(dt_ + 1) * P],
                                            ident)
                    if dt_ % 2 == 0:
                        nc.scalar.copy(x2T[:, dt_, cp * 2 * P:(cp + 1) * 2 * P], pt)
                    else:
                        nc.vector.tensor_copy(x2T[:, dt_, cp * 2 * P:(cp + 1) * 2 * P], pt)

        for tt in range(NO):
            nc.vector.tensor_add(res1[:, tt, :], res1[:, tt, :], b2_sb)

        ps_tt = [pffn.tile([P, 512], f32, tag=f"pf{tt}", name=f"pf{tt}") for tt in range(NO)]

        def emit_f2(hq):
            for tt in range(NO):
                mm(ps_tt[tt], g_sb[:, 2 * hq:2 * hq + 2, tt * P:(tt + 1) * P],
                   w2_sb[:, 2 * hq:2 * hq + 2, :],
                   start=(hq == 0), stop=(hq == NHID // 2 - 1), perf_mode=DR)

        for ht in range(NHID):
            ps = pmm_t.tile([P, 512], f32, tag="ps")
            for dp in range(2):
                mm(ps, w1_sb[:, 2 * dp:2 * dp + 2, ht * P:(ht + 1) * P],
                   x2T[:, 2 * dp:2 * dp + 2, :],
                   start=(dp == 0), stop=(dp == 1), perf_mode=DR)
            nc.scalar.activation(g_sb[:, ht, :], ps, AF.Gelu,
                                 bias=b1_sb[:, ht:ht + 1], scale=1.0 / WS)
            if ht >= 3 and ht % 2 == 1:
                emit_f2(ht // 2 - 1)
        emit_f2(NHID // 2 - 1)
        for tt in range(NO):
            nc.vector.scalar_tensor_tensor(out=fin[:, tt, :], in0=ps_tt[tt],
                                           scalar=1.0 / WS, in1=res1[:, tt, :],
                                           op0=OP.mult, op1=OP.add)

        yr = y.rearrange("(j p) d -> p j d", p=P)
        for tt in range(NO):
            nc.sync.dma_start(out=yr[:, tt, :], in_=fin[:, tt, :])


def _build():
    from contextlib import ExitStack

    import concourse.bacc as bacc
    import concourse.tile as tile
    from concourse import mybir

    f32 = mybir.dt.float32
    bf16 = mybir.dt.bfloat16
    f8 = mybir.dt.float8e4
    nc = bacc.Bacc("TRN2", target_bir_lowering=False, debug=False,
                   enable_asserts=False, num_devices=NCORES)
    I = {}

    def inp(name, shape, dt_):
        I[name] = nc.dram_tensor(name, list(shape), dt_, kind="ExternalInput").ap()

    inp("xc", (P, NT, D), f32)
    inp("ident", (P, P), bf16)
    inp("wqkvT", (P, ND, 3 * D), f8)
    inp("bq", (P, 4), f32)
    inp("bk", (P, 4), f32)
    inp("woT", (P, ND, D), f8)
    inp("bo", (D,), f32)
    inp("w1T", (P, ND, HIDDEN), f8)
    inp("b1", (P, NHID), f32)
    inp("w2T", (P, NHID, D), f8)
    inp("b2", (D,), f32)
    inp("masks", (P, 2, 2 * SQ), bf16)
    y = nc.dram_tensor("y", [CH, D], f32, kind="ExternalOutput").ap()

    with tile.TileContext(nc) as tc:
        with ExitStack() as ctx:
            _body(ctx, tc, I, y)
    nc.compile()
    return nc


def _host_masks():
    """Additive pre-softmax masks: 0 where valid, -240 where masked
    (exp(-240/8) == 0 in bf16). Added into the S psum with an
    ident-lhsT matmul instead of multiplying after exp."""
    import ml_dtypes
    tril = np.tril(np.ones((P, P), np.float32))
    triu = np.triu(np.ones((P, P), np.float32))
    kt02 = np.concatenate([tril, tril, triu, triu], 1)
    kt1 = np.concatenate([triu, tril, triu, tril], 1)
    m = np.stack([kt02, kt1])
    m0 = m.copy()
    m0[0, :, 0:256] = 0.0  # first chunk of each batch: halo keys invalid
    m = ((m - 1.0) * 240.0).astype(ml_dtypes.bfloat16)
    m0 = ((m0 - 1.0) * 240.0).astype(ml_dtypes.bfloat16)
    m = np.ascontiguousarray(m.transpose(1, 0, 2))
    m0 = np.ascontiguousarray(m0.transpose(1, 0, 2))
    return m, m0


def get_nc():
    global _nc
    if _nc is None:
        _nc = _build()
    return _nc


def _pmaj(a, p=P):
    """[N*p, F...] row-major -> [p, N, F...] partition-major contiguous."""
    n = a.shape[0] // p
    return np.ascontiguousarray(
        a.reshape((n, p) + a.shape[1:]).transpose((1, 0) + tuple(range(2, a.ndim + 1))))


# stream-major chunk gather: full[768] = [halo 256 | own 512]
_a = np.arange(128)
_XIDX = np.concatenate([256 + 2 * _a, 512 + 2 * _a, 257 + 2 * _a,
                        513 + 2 * _a, 2 * _a, 1 + 2 * _a])
# y rows (chunk-major stream order) -> original own-token index
_TOKPERM = np.concatenate([2 * _a, 256 + 2 * _a, 1 + 2 * _a, 257 + 2 * _a])


def make_in_maps(inputs):
    import ml_dtypes
    f = np.float32
    bf = ml_dtypes.bfloat16
    f8 = ml_dtypes.float8_e4m3
    x = np.asarray(inputs["x"], f)
    qkv_w = np.asarray(inputs["qkv_w"], f)
    n1w = np.asarray(inputs["norm1_w"], f)
    n1b = np.asarray(inputs["norm1_b"], f)
    wqkv_f = qkv_w * n1w[None, :]
    bqkv = qkv_w @ n1b + np.asarray(inputs["qkv_b"], f)
    wqkvT = _pmaj(np.ascontiguousarray(wqkv_f.T) * WS).astype(f8)
    bq = np.ascontiguousarray(bqkv[0:D].reshape(4, P).T)
    bk = np.ascontiguousarray(bqkv[D:2 * D].reshape(4, P).T)
    bv = np.ascontiguousarray(bqkv[2 * D:3 * D])

    out_w = np.asarray(inputs["out_w"], f)
    woT = _pmaj(np.ascontiguousarray(out_w.T) * WS).astype(f8)
    bo = np.ascontiguousarray(out_w @ bv + np.asarray(inputs["out_b"], f))

    w1 = np.asarray(inputs["ffn_w1"], f)
    n2w = np.asarray(inputs["norm2_w"], f)
    n2b = np.asarray(inputs["norm2_b"], f)
    w1T = _pmaj(np.ascontiguousarray((w1 * n2w[None, :]).T) * WS).astype(f8)
    b1v = w1 @ n2b + np.asarray(inputs["ffn_b1"], f)
    b1 = np.ascontiguousarray(b1v.reshape(NHID, P).T)
    w2T = _pmaj(np.ascontiguousarray(np.asarray(inputs["ffn_w2"], f).T) * WS).astype(f8)
    b2 = np.ascontiguousarray(np.asarray(inputs["ffn_b2"], f))

    ident = np.eye(P, dtype=bf)
    masks, masks0 = _host_masks()
    shared = dict(ident=ident, wqkvT=wqkvT, bq=bq, bk=bk, woT=woT, bo=bo,
                  w1T=w1T, b1=b1, w2T=w2T, b2=b2)
    in_maps = []
    for c in range(NCORES):
        b_, i = divmod(c, 4)
        own = x[b_, i * CH:(i + 1) * CH]
        if i == 0:
            halo = np.zeros((HALO, D), f)
        else:
            halo = x[b_, i * CH - HALO:i * CH]
        full = np.concatenate([halo, own], 0)
        xc = np.ascontiguousarray(
            full[_XIDX].reshape(NT, P, D).transpose(1, 0, 2))
        in_maps.append(dict(xc=xc, masks=(masks if i > 0 else masks0), **shared))
    return in_maps


def kernel(**inputs):
    global LAST_EXEC_NS, LAST_RESULTS
    from concourse.bass_utils import run_bass_kernel_spmd

    nc = get_nc()
    in_maps = make_in_maps(inputs)
    trace = bool(int(os.environ.get("BASS_KERNEL_TRACE", "0")))
    res = run_bass_kernel_spmd(nc, in_maps, core_ids=list(range(NCORES)),
                               trace=trace)
    LAST_EXEC_NS = res.exec_time_ns
    LAST_RESULTS = res
    out = np.zeros((B, L, D), np.float32)
    for c, r in enumerate(res.results):
        b_, i = divmod(c, 4)
        out[b_, i * CH + _TOKPERM] = r["y"]
    return out
